# revision 9
# baseline (speedup 1.0000x reference)
"""Trainium2 Bass kernel for EnhancedBertForIdiomDetection (SPMD, 8 cores).

Sharding:
  - BERT encoder + head: data parallel, 4 seqs (1024 tokens) per core.
  - BiLSTM: 8 cores = 2 directions x 4 seq-groups (8 seqs per core); the
    backward direction's time reversal is handled by per-core gather index
    tensors so every core runs the identical SPMD program.
  - CRF logZ / gold score / viterbi: full batch (32 seqs on partitions),
    computed redundantly on every core after an AllGather of emissions.
BERT activations are kept transposed ([feature, token]) so the matmul chain
needs no activation transposes; LayerNorm over the feature (partition) dim
uses TensorE ones-reductions.
"""
import sys

sys.path.insert(0, "/opt/trn_rl_repo")

import numpy as np

V, SEQ, BATCH = 119547, 256, 32
D, NL, NH, DH, FF = 768, 12, 12, 64, 3072
HL, NUM_LABELS = 384, 5
NCORES = 8
TOK = 4 * SEQ
LTOK = 8 * SEQ
DC = D // 128
FC = FF // 128
HC = (2 * HL) // 128
ZC = HL // 128
G4 = 4 * HL
GC = G4 // 128
LB = 8

_CACHE = {}


def _build(nl, run_lstm, run_head):
    import concourse.bass as bass_mod
    import concourse.mybir as mybir
    import concourse.tile as tile
    from concourse import bacc
    from concourse.masks import make_identity

    F32 = mybir.dt.float32
    F16 = mybir.dt.float16
    I32 = mybir.dt.int32
    AF = mybir.ActivationFunctionType
    ALU = mybir.AluOpType
    AX = mybir.AxisListType.X
    IOff = bass_mod.IndirectOffsetOnAxis

    nc = bacc.Bacc(None)
    dp = lambda name, shape, dt=F32: nc.declare_dram_parameter(name, list(shape), dt, isOutput=False)
    do = lambda name, shape, dt=F32: nc.declare_dram_parameter(name, list(shape), dt, isOutput=True)

    ids = dp("ids", [TOK, 1], I32)
    wemb = dp("wemb", [V, D])
    pos_type = dp("pos_type", [SEQ, D])
    embg = dp("embg", [1, D])
    embb = dp("embb", [1, D])
    LW = []
    for l in range(nl):
        LW.append(dict(
            wq=dp(f"L{l}_wq", [D, D]), wk=dp(f"L{l}_wk", [D, D]),
            wv=dp(f"L{l}_wv", [D, D]), wo=dp(f"L{l}_wo", [D, D]),
            bqkv=dp(f"L{l}_bqkv", [128, 3 * DC]), bo=dp(f"L{l}_bo", [128, DC]),
            ln1g=dp(f"L{l}_ln1g", [128, DC]), ln1b=dp(f"L{l}_ln1b", [128, DC]),
            w1=dp(f"L{l}_w1", [D, FF]), b1=dp(f"L{l}_b1", [128, FC]),
            w2=dp(f"L{l}_w2", [FF, D]), b2=dp(f"L{l}_b2", [128, DC]),
            ln2g=dp(f"L{l}_ln2g", [128, DC]), ln2b=dp(f"L{l}_ln2b", [128, DC]),
        ))
    if run_lstm:
        lidx1 = dp("lidx1", [LTOK, 1], I32)
        lidx2F = dp("lidx2F", [LTOK, 1], I32)
        lidx2B = dp("lidx2B", [LTOK, 1], I32)
        wihs = [dp(f"wih{ll}", [2 * HL, G4]) for ll in range(2)]
        whhp = [dp(f"whh{ll}", [128, 3 * G4], F16) for ll in range(2)]
        lbc = [dp(f"lbc{ll}", [128, GC]) for ll in range(2)]
    if run_head:
        hidxF = dp("hidxF", [TOK, 1], I32)
        hidxB = dp("hidxB", [TOK, 1], I32)
        dense_w = dp("dense_w", [2 * HL, HL])
        dense_b = dp("dense_b", [128, ZC])
        normg = dp("normg", [128, ZC])
        normb = dp("normb", [128, ZC])
        cls_w = dp("cls_w", [HL, NUM_LABELS])
        cls_b = dp("cls_b", [1, NUM_LABELS])
        pool_w1 = dp("pool_w1", [HL, HL])
        pool_b1 = dp("pool_b1", [128, ZC])
        pool_w2 = dp("pool_w2", [HL, 2])
        pool_b2 = dp("pool_b2", [2, 1])
        crf_start = dp("crf_start", [1, 5])
        crf_end = dp("crf_end", [1, 5])
        crf_trans = dp("crf_trans", [1, 25])
        iota5 = dp("iota5", [1, 5])
        iota5m8 = dp("iota5m8", [1, 5])
        iota2 = dp("iota2", [1, 2])
        labels_full = dp("labels_full", [BATCH, SEQ], I32)
        mask_full = dp("mask_full", [BATCH, SEQ], I32)
        slab_full = dp("slab_full", [BATCH, 1], I32)

    out_x = do("out_x", [TOK, D])
    if run_head:
        out_emis = do("out_emis", [TOK, NUM_LABELS])
        out_slg = do("out_slg", [4, 2])
        out_preds = do("out_preds", [BATCH, SEQ])
        out_loss = do("out_loss", [1, 1])

    xout_d = nc.dram_tensor("xout_d", [TOK, D], F32)
    if run_lstm:
        xag = nc.dram_tensor("xag", [NCORES * TOK, D], F32, addr_space="Shared")
        x2T_d = nc.dram_tensor("x2T_d", [128, DC * LTOK], F32)
        pre_d = nc.dram_tensor("pre_d", [128, SEQ * GC * LB], F32)
        h_loc = [nc.dram_tensor(f"h{ll}loc", [LB * SEQ, HL], F32) for ll in range(2)]
        h_ag = [nc.dram_tensor(f"h{ll}ag", [NCORES * LB * SEQ, HL], F32, addr_space="Shared")
                for ll in range(2)]
    if run_head:
        emis_d = nc.dram_tensor("emis_d", [TOK, NUM_LABELS], F32)
        emis_ag = nc.dram_tensor("emis_ag", [NCORES * TOK, NUM_LABELS], F32, addr_space="Shared")
        slg_d = nc.dram_tensor("slg_d", [4, 2], F32)
        slg_ag = nc.dram_tensor("slg_ag", [BATCH, 2], F32, addr_space="Shared")

    GRP = [list(range(NCORES))]

    with tile.TileContext(nc) as tc:
        import contextlib
        with contextlib.ExitStack() as ctx:
            gp = ctx.enter_context(tc.tile_pool(name="gp", bufs=1))
            rows = ctx.enter_context(tc.tile_pool(name="rows", bufs=3))
            wt = ctx.enter_context(tc.tile_pool(name="wt", bufs=8))
            psp = ctx.enter_context(tc.tile_pool(name="psp", bufs=1, space="PSUM"))

            ident = gp.tile([128, 128], F32)
            make_identity(nc, ident[:])
            ones_col = gp.tile([128, 1], F32)
            nc.vector.memset(ones_col[:], 1.0)
            eps_col = gp.tile([128, 1], F32)
            nc.vector.memset(eps_col[:], 1e-12)

            def dma(out, in_):
                nc.sync.dma_start(out=out, in_=in_)

            def load_cols(pool, dram_t, ncols, tag):
                t = pool.tile([128, ncols], F32, tag=tag, name=tag)
                dma(t[:], dram_t[:, 0:ncols])
                return t

            def ln_T(sc, dst, src, nch, ntok, g_col, b_col, dim):
                half = ntok // 2
                sums = rows.tile([1, ntok], F32, tag="lnrow", name="sums")
                sums2 = rows.tile([1, ntok], F32, tag="lnrow", name="sums2")
                for th in range(2):
                    p = psp.tile([1, half], F32, tag="A", bufs=2, name="lnp")
                    for c in range(nch):
                        nc.tensor.matmul(out=p[:], lhsT=ones_col[:],
                                         rhs=src[:, c * ntok + th * half: c * ntok + (th + 1) * half],
                                         start=(c == 0), stop=(c == nch - 1))
                    nc.scalar.copy(sums[:, th * half:(th + 1) * half], p[:])
                for th in range(2):
                    p = psp.tile([1, half], F32, tag="A", bufs=2, name="lnp2")
                    for c in range(nch):
                        sq = sc.tile([128, half], F32, tag="lnsq", name="sq")
                        nc.scalar.square(sq[:], src[:, c * ntok + th * half: c * ntok + (th + 1) * half])
                        nc.tensor.matmul(out=p[:], lhsT=ones_col[:], rhs=sq[:],
                                         start=(c == 0), stop=(c == nch - 1))
                    nc.scalar.copy(sums2[:, th * half:(th + 1) * half], p[:])
                mu = rows.tile([1, ntok], F32, tag="lnrow", name="mu")
                nc.vector.tensor_scalar(mu[:], sums[:], 1.0 / dim, None, op0=ALU.mult)
                var = rows.tile([1, ntok], F32, tag="lnrow", name="var")
                nc.vector.tensor_tensor(var[:], mu[:], mu[:], op=ALU.mult)
                nc.vector.scalar_tensor_tensor(var[:], sums2[:], 1.0 / dim, var[:],
                                               op0=ALU.mult, op1=ALU.subtract)
                srt = rows.tile([1, ntok], F32, tag="lnrow", name="srt")
                nc.scalar.activation(srt[:], var[:], AF.Sqrt, bias=eps_col[0:1, 0:1], scale=1.0)
                rstd = rows.tile([1, ntok], F32, tag="lnrow", name="rstd")
                nc.vector.reciprocal(rstd[:], srt[:])
                mub = sc.tile([128, ntok], F32, tag="lnbc", name="mub")
                rstdb = sc.tile([128, ntok], F32, tag="lnbc", name="rstdb")
                nc.gpsimd.partition_broadcast(mub[:], mu[:])
                nc.gpsimd.partition_broadcast(rstdb[:], rstd[:])
                for c in range(nch):
                    t1 = sc.tile([128, ntok], F32, tag="lnt1", name="t1")
                    nc.vector.tensor_sub(t1[:], src[:, c * ntok:(c + 1) * ntok], mub[:])
                    nc.vector.tensor_mul(t1[:], t1[:], rstdb[:])
                    nc.scalar.activation(dst[:, c * ntok:(c + 1) * ntok], t1[:],
                                         AF.Identity, bias=b_col[:, c:c + 1],
                                         scale=g_col[:, c:c + 1])

            def proj_T(dst, src, w_dram, nin_ch, nout_ch, ntok, bias_col=None,
                       func=None, badd=None):
                func = func if func is not None else AF.Identity
                half = 512
                for n in range(nout_ch):
                    for th in range(ntok // half):
                        p = psp.tile([128, half], F32, tag="A", bufs=2, name="pp")
                        for k in range(nin_ch):
                            w = wt.tile([128, 128], F32, tag="w", name="w")
                            dma(w[:], w_dram[k * 128:(k + 1) * 128, n * 128:(n + 1) * 128])
                            nc.tensor.matmul(out=p[:], lhsT=w[:],
                                             rhs=src[:, k * ntok + th * half: k * ntok + th * half + half],
                                             start=(k == 0), stop=(k == nin_ch - 1))
                        dsl = dst[:, n * ntok + th * half: n * ntok + th * half + half]
                        if badd is not None:
                            nc.vector.scalar_tensor_tensor(
                                dsl, p[:], bias_col[:, n:n + 1],
                                badd[:, n * ntok + th * half: n * ntok + th * half + half],
                                op0=ALU.add, op1=ALU.add)
                        else:
                            nc.scalar.activation(dsl, p[:], func,
                                                 bias=0.0 if bias_col is None else bias_col[:, n:n + 1],
                                                 scale=1.0)

            def gather_transpose_T(sc, dst_writer, src_dram, idx_sb, ntiles, nch, extra=None):
                for t in range(ntiles):
                    g = sc.tile([128, nch * 128], F32, tag="gath", name="g")
                    if extra is None:
                        nc.gpsimd.indirect_dma_start(out=g[:], out_offset=None, in_=src_dram[:],
                                                     in_offset=IOff(ap=idx_sb[:, t:t + 1], axis=0))
                    else:
                        src2, idx2, hw = extra
                        nc.gpsimd.indirect_dma_start(out=g[:, 0:hw], out_offset=None, in_=src_dram[:],
                                                     in_offset=IOff(ap=idx_sb[:, t:t + 1], axis=0))
                        nc.gpsimd.indirect_dma_start(out=g[:, hw:2 * hw], out_offset=None, in_=src2[:],
                                                     in_offset=IOff(ap=idx2[:, t:t + 1], axis=0))
                    for c in range(nch):
                        pt = psp.tile([128, 128], F32, tag="A", bufs=2, name="pt")
                        nc.tensor.transpose(out=pt[:], in_=g[:, c * 128:(c + 1) * 128], identity=ident[:])
                        dst_writer(t, c, pt)

            # ============================================================
            # Phase 1: embedding + BERT
            # ============================================================
            with tc.tile_pool(name="bact", bufs=1) as bact, \
                 tc.tile_pool(name="bsc", bufs=2) as bsc, \
                 tc.tile_pool(name="bcon", bufs=1) as bcon:
                ids_sb = bcon.tile([128, 8], I32)
                dma(ids_sb[:], ids[:].rearrange("(t p) o -> p (t o)", p=128))
                pos_sb = bcon.tile([128, 2 * D], F32)
                dma(pos_sb[:, 0:D], pos_type[0:128, :])
                dma(pos_sb[:, D:2 * D], pos_type[128:256, :])
                embg_b = bcon.tile([128, D], F32)
                embb_b = bcon.tile([128, D], F32)
                dma(embg_b[:], embg[:].to_broadcast([128, D]))
                dma(embb_b[:], embb[:].to_broadcast([128, D]))

                xT = bact.tile([128, DC * 1024], F32, tag="xT", name="xT")
                for t in range(8):
                    g = bsc.tile([128, D], F32, tag="emb", name="g")
                    nc.gpsimd.indirect_dma_start(out=g[:], out_offset=None, in_=wemb[:],
                                                 in_offset=IOff(ap=ids_sb[:, t:t + 1], axis=0))
                    nc.vector.tensor_add(g[:], g[:], pos_sb[:, (t % 2) * D:(t % 2 + 1) * D])
                    nmu = bsc.tile([128, 1], F32, tag="embs", name="nmu")
                    nc.vector.reduce_sum(nmu[:], g[:], axis=AX, negate=True)
                    nc.vector.tensor_scalar(nmu[:], nmu[:], 1.0 / D, None, op0=ALU.mult)
                    xc = bsc.tile([128, D], F32, tag="emb2", name="xc")
                    nc.vector.tensor_scalar(xc[:], g[:], nmu[:, 0:1], None, op0=ALU.add)
                    sqv = bsc.tile([128, D], F32, tag="emb3", name="sqv")
                    ssq = bsc.tile([128, 1], F32, tag="embs", name="ssq")
                    nc.scalar.activation(sqv[:], xc[:], AF.Square, accum_out=ssq[:, 0:1])
                    srt = bsc.tile([128, 1], F32, tag="embs", name="srt")
                    nc.scalar.activation(srt[:], ssq[:], AF.Sqrt, bias=eps_col[:, 0:1], scale=1.0 / D)
                    rstd = bsc.tile([128, 1], F32, tag="embs", name="rstd")
                    nc.vector.reciprocal(rstd[:], srt[:])
                    nc.vector.tensor_scalar(xc[:], xc[:], rstd[:, 0:1], None, op0=ALU.mult)
                    nc.vector.tensor_mul(xc[:], xc[:], embg_b[:])
                    nc.vector.tensor_add(xc[:], xc[:], embb_b[:])
                    for c in range(DC):
                        pt = psp.tile([128, 128], F32, tag="A", bufs=2, name="pt")
                        nc.tensor.transpose(out=pt[:], in_=xc[:, c * 128:(c + 1) * 128], identity=ident[:])
                        nc.scalar.copy(xT[:, c * 1024 + t * 128: c * 1024 + (t + 1) * 128], pt[:])

                for l in range(nl):
                    W = LW[l]
                    bqkv_c = load_cols(bcon, W["bqkv"], 3 * DC, "c_bqkv")
                    bo_c = load_cols(bcon, W["bo"], DC, "c_bo")
                    ln1g_c = load_cols(bcon, W["ln1g"], DC, "c_l1g")
                    ln1b_c = load_cols(bcon, W["ln1b"], DC, "c_l1b")
                    b1_c = load_cols(bcon, W["b1"], FC, "c_b1")
                    b2_c = load_cols(bcon, W["b2"], DC, "c_b2")
                    ln2g_c = load_cols(bcon, W["ln2g"], DC, "c_l2g")
                    ln2b_c = load_cols(bcon, W["ln2b"], DC, "c_l2b")
                    qT = bact.tile([128, DC * 1024], F32, tag="qT", name="qT")
                    kT = bact.tile([128, DC * 1024], F32, tag="kT", name="kT")
                    vT = bact.tile([128, DC * 1024], F32, tag="vT", name="vT")
                    for mi, (wm, tgt) in enumerate([(W["wq"], qT), (W["wk"], kT), (W["wv"], vT)]):
                        proj_T(tgt, xT, wm, DC, DC, 1024,
                               bias_col=bqkv_c[:, mi * DC:(mi + 1) * DC])
                    vnat = bact.tile([128, 8 * D], F32, tag="vnat", name="vnat")
                    for t in range(8):
                        for c in range(DC):
                            pt = psp.tile([128, 128], F32, tag="A", bufs=2, name="pt")
                            nc.tensor.transpose(out=pt[:], in_=vT[:, c * 1024 + t * 128: c * 1024 + (t + 1) * 128],
                                                identity=ident[:])
                            nc.scalar.copy(vnat[:, t * D + c * 128: t * D + (c + 1) * 128], pt[:])
                    ctxT = bact.tile([128, DC * 1024], F32, tag="vT", name="ctxT")
                    for s in range(4):
                        for h in range(NH):
                            hc, po = h // 2, (h % 2) * DH
                            pT = bsc.tile([128, 512], F32, tag="attp", name="pT")
                            for qc in range(2):
                                pS = psp.tile([128, 256], F32, tag="B", bufs=6, name="pS")
                                nc.tensor.matmul(
                                    out=pS[:],
                                    lhsT=qT[po:po + DH, hc * 1024 + s * 256 + qc * 128: hc * 1024 + s * 256 + (qc + 1) * 128],
                                    rhs=kT[po:po + DH, hc * 1024 + s * 256: hc * 1024 + (s + 1) * 256],
                                    start=True, stop=True)
                                nm = bsc.tile([128, 1], F32, tag="atts", name="nm")
                                nc.vector.reduce_max(nm[:], pS[:], axis=AX, negate=True)
                                nc.vector.tensor_scalar(nm[:], nm[:], 0.125, None, op0=ALU.mult)
                                ex = bsc.tile([128, 256], F32, tag="attx", name="ex")
                                sume = bsc.tile([128, 1], F32, tag="atts", name="sume")
                                nc.scalar.activation(ex[:], pS[:], AF.Exp, bias=nm[:, 0:1],
                                                     scale=0.125, accum_out=sume[:, 0:1])
                                rs = bsc.tile([128, 1], F32, tag="atts", name="rs")
                                nc.vector.reciprocal(rs[:], sume[:])
                                nc.vector.tensor_scalar(ex[:], ex[:], rs[:, 0:1], None, op0=ALU.mult)
                                for kc in range(2):
                                    pt = psp.tile([128, 128], F32, tag="A", bufs=2, name="pt")
                                    nc.tensor.transpose(out=pt[:], in_=ex[:, kc * 128:(kc + 1) * 128],
                                                        identity=ident[:])
                                    nc.scalar.copy(pT[:, kc * 256 + qc * 128: kc * 256 + (qc + 1) * 128], pt[:])
                            pc = psp.tile([64, 256], F32, tag="B", bufs=6, name="pc")
                            for kc in range(2):
                                nc.tensor.matmul(
                                    out=pc[:],
                                    lhsT=vnat[:, (2 * s + kc) * D + h * DH: (2 * s + kc) * D + (h + 1) * DH],
                                    rhs=pT[:, kc * 256:(kc + 1) * 256],
                                    start=(kc == 0), stop=(kc == 1))
                            nc.scalar.copy(ctxT[po:po + DH, hc * 1024 + s * 256: hc * 1024 + (s + 1) * 256], pc[:])
                    y1 = bact.tile([128, DC * 1024], F32, tag="qT", name="y1")
                    proj_T(y1, ctxT, W["wo"], DC, DC, 1024, bias_col=bo_c, badd=xT)
                    x1T = bact.tile([128, DC * 1024], F32, tag="kT", name="x1T")
                    ln_T(bsc, x1T, y1, DC, 1024, ln1g_c, ln1b_c, D)
                    y2 = bact.tile([128, DC * 1024], F32, tag="qT", name="y2")
                    for th in range(2):
                        pouts = []
                        for pi in range(DC):
                            po_t = psp.tile([128, 512], F32, tag="B", bufs=6, name=f"pout{pi}")
                            pouts.append(po_t)
                        for hcc in range(FC):
                            ph = psp.tile([128, 512], F32, tag="A", bufs=2, name="ph")
                            for k in range(DC):
                                w = wt.tile([128, 128], F32, tag="w", name="w1t")
                                dma(w[:], W["w1"][k * 128:(k + 1) * 128, hcc * 128:(hcc + 1) * 128])
                                nc.tensor.matmul(out=ph[:], lhsT=w[:],
                                                 rhs=x1T[:, k * 1024 + th * 512: k * 1024 + th * 512 + 512],
                                                 start=(k == 0), stop=(k == DC - 1))
                            hsb = bsc.tile([128, 512], F32, tag="ffa", name="hsb")
                            nc.scalar.activation(hsb[:], ph[:], AF.Gelu, bias=b1_c[:, hcc:hcc + 1], scale=1.0)
                            for n in range(DC):
                                w = wt.tile([128, 128], F32, tag="w", name="w2t")
                                dma(w[:], W["w2"][hcc * 128:(hcc + 1) * 128, n * 128:(n + 1) * 128])
                                nc.tensor.matmul(out=pouts[n][:], lhsT=w[:], rhs=hsb[:],
                                                 start=(hcc == 0), stop=(hcc == FC - 1))
                        for n in range(DC):
                            nc.vector.scalar_tensor_tensor(
                                y2[:, n * 1024 + th * 512: n * 1024 + th * 512 + 512],
                                pouts[n][:], b2_c[:, n:n + 1],
                                x1T[:, n * 1024 + th * 512: n * 1024 + th * 512 + 512],
                                op0=ALU.add, op1=ALU.add)
                    xT = bact.tile([128, DC * 1024], F32, tag="xT", name="xTn")
                    ln_T(bsc, xT, y2, DC, 1024, ln2g_c, ln2b_c, D)

                for t in range(8):
                    xo = bsc.tile([128, D], F32, tag="emb", name="xo")
                    for c in range(DC):
                        pt = psp.tile([128, 128], F32, tag="A", bufs=2, name="pt")
                        nc.tensor.transpose(out=pt[:], in_=xT[:, c * 1024 + t * 128: c * 1024 + (t + 1) * 128],
                                            identity=ident[:])
                        nc.scalar.copy(xo[:, c * 128:(c + 1) * 128], pt[:])
                    dma(xout_d[t * 128:(t + 1) * 128, :], xo[:])
                    dma(out_x[t * 128:(t + 1) * 128, :], xo[:])

            # ============================================================
            # Phase 2: BiLSTM
            # ============================================================
            if run_lstm:
                nc.gpsimd.collective_compute("AllGather", ALU.bypass, replica_groups=GRP,
                                             ins=[xout_d[:]], outs=[xag[:]])
                x2T3 = x2T_d[:].rearrange("p (c r) -> p c r", r=LTOK)
                pre3 = pre_d[:].rearrange("p (t q) -> p t q", q=GC * LB)

                with tc.tile_pool(name="lsc", bufs=2) as lsc, \
                     tc.tile_pool(name="lcon", bufs=1) as lcon:
                    def lstm_layer(ll, src1, src2, idxF_d, idxB_d, h_loc_d):
                        idxF_sb = lcon.tile([128, 16], I32, tag="idxF", name="idxF")
                        dma(idxF_sb[:], idxF_d[:].rearrange("(t p) o -> p (t o)", p=128))
                        idxB_sb = None
                        if idxB_d is not None:
                            idxB_sb = lcon.tile([128, 16], I32, tag="idxB", name="idxB")
                            dma(idxB_sb[:], idxB_d[:].rearrange("(t p) o -> p (t o)", p=128))

                        def wr(t, c, pt):
                            stg = lsc.tile([128, 128], F32, tag="ltstg", name="stg")
                            nc.scalar.copy(stg[:], pt[:])
                            dma(x2T3[:, c:c + 1, t * 128:(t + 1) * 128].rearrange("p c r -> p (c r)"),
                                stg[:])

                        gather_transpose_T(lsc, wr, src1, idxF_sb, 16, DC,
                                           extra=None if idxB_d is None else (src2, idxB_sb, HL))
                        lbc_c = load_cols(lcon, lbc[ll], GC, "c_lbc")
                        for tc4 in range(4):
                            x2s = lsc.tile([128, DC * 512], F32, tag="x2s", name="x2s")
                            dma(x2s[:], x2T3[:, :, tc4 * 512:(tc4 + 1) * 512])
                            for n in range(GC):
                                p = psp.tile([128, 512], F32, tag="A", bufs=2, name="pp")
                                for k in range(DC):
                                    w = wt.tile([128, 128], F32, tag="w", name="wihw")
                                    dma(w[:], wihs[ll][k * 128:(k + 1) * 128, n * 128:(n + 1) * 128])
                                    nc.tensor.matmul(out=p[:], lhsT=w[:],
                                                     rhs=x2s[:, k * 512:(k + 1) * 512],
                                                     start=(k == 0), stop=(k == DC - 1))
                                stg = lsc.tile([128, 512], F32, tag="prestg", name="pstg")
                                nc.scalar.activation(stg[:], p[:], AF.Identity,
                                                     bias=lbc_c[:, n:n + 1], scale=1.0)
                                dma(pre3[:, tc4 * 64:(tc4 + 1) * 64, n * LB:(n + 1) * LB],
                                    stg[:].rearrange("p (t s) -> p t s", s=LB))
                        whh_sb = lcon.tile([128, 3 * G4], F16, tag="whh", name="whh")
                        dma(whh_sb[:], whhp[ll][:])
                        h16 = lsc.tile([128, 3 * LB], F16, tag="h16", name="h16i")
                        cst = lsc.tile([128, 3 * LB], F32, tag="cst", name="csti")
                        nc.vector.memset(h16[:], 0.0)
                        nc.vector.memset(cst[:], 0.0)
                        BLK = 16
                        for blk in range(SEQ // BLK):
                            preb = lsc.tile([128, BLK * GC * LB], F32, tag="preblk", name="preb")
                            dma(preb[:], pre3[:, blk * BLK:(blk + 1) * BLK, :])
                            hist = lsc.tile([128, BLK * 3 * LB], F32, tag="hist", name="hist")
                            for tl in range(BLK):
                                pg = psp.tile([128, GC * LB], F32, tag="A", bufs=2, name="pg")
                                for n in range(GC):
                                    for k in range(3):
                                        nc.tensor.matmul(
                                            out=pg[:, n * LB:(n + 1) * LB],
                                            lhsT=whh_sb[:, (k * GC + n) * 128:(k * GC + n + 1) * 128],
                                            rhs=h16[:, k * LB:(k + 1) * LB],
                                            start=(k == 0), stop=(k == 2))
                                gsb = lsc.tile([128, GC * LB], F32, tag="gsb", name="gsb")
                                nc.vector.tensor_add(gsb[:], pg[:], preb[:, tl * GC * LB:(tl + 1) * GC * LB])
                                sif = lsc.tile([128, 6 * LB], F32, tag="sif", name="sif")
                                nc.scalar.activation(sif[:], gsb[:, 0:6 * LB], AF.Sigmoid)
                                tg = lsc.tile([128, 3 * LB], F32, tag="tg", name="tg")
                                nc.scalar.activation(tg[:], gsb[:, 6 * LB:9 * LB], AF.Tanh)
                                so = lsc.tile([128, 3 * LB], F32, tag="so", name="so")
                                nc.scalar.activation(so[:], gsb[:, 9 * LB:12 * LB], AF.Sigmoid)
                                c2 = lsc.tile([128, 3 * LB], F32, tag="cst", name="c2")
                                nc.vector.tensor_mul(c2[:], sif[:, 3 * LB:6 * LB], cst[:])
                                t2 = lsc.tile([128, 3 * LB], F32, tag="t2", name="t2")
                                nc.vector.tensor_mul(t2[:], sif[:, 0:3 * LB], tg[:])
                                nc.vector.tensor_add(c2[:], c2[:], t2[:])
                                cst = c2
                                tch = lsc.tile([128, 3 * LB], F32, tag="tch", name="tch")
                                nc.scalar.activation(tch[:], cst[:], AF.Tanh)
                                hsl = hist[:, tl * 3 * LB:(tl + 1) * 3 * LB]
                                nc.vector.tensor_mul(hsl, so[:], tch[:])
                                h16 = lsc.tile([128, 3 * LB], F16, tag="h16", name="h16")
                                nc.vector.tensor_copy(h16[:], hsl)
                            hist3 = hist[:].rearrange("p (t q) -> p t q", q=3 * LB)
                            for c in range(3):
                                for s in range(LB):
                                    dma(h_loc_d[s * SEQ + blk * BLK: s * SEQ + (blk + 1) * BLK,
                                                c * 128:(c + 1) * 128].rearrange("t (f o) -> f t o", o=1),
                                        hist3[:, :, c * LB + s: c * LB + s + 1])

                    lstm_layer(0, xag, None, lidx1, None, h_loc[0])
                    nc.gpsimd.collective_compute("AllGather", ALU.bypass, replica_groups=GRP,
                                                 ins=[h_loc[0][:]], outs=[h_ag[0][:]])
                    lstm_layer(1, h_ag[0], h_ag[0], lidx2F, lidx2B, h_loc[1])
                    nc.gpsimd.collective_compute("AllGather", ALU.bypass, replica_groups=GRP,
                                                 ins=[h_loc[1][:]], outs=[h_ag[1][:]])

            # ============================================================
            # Phase 3: head + full-batch CRF
            # ============================================================
            if run_head:
                with tc.tile_pool(name="hact", bufs=1) as hact, \
                     tc.tile_pool(name="hsc", bufs=2) as hsc, \
                     tc.tile_pool(name="hcon", bufs=1) as hcon:
                    hidxF_sb = hcon.tile([128, 8], I32)
                    dma(hidxF_sb[:], hidxF[:].rearrange("(t p) o -> p (t o)", p=128))
                    hidxB_sb = hcon.tile([128, 8], I32)
                    dma(hidxB_sb[:], hidxB[:].rearrange("(t p) o -> p (t o)", p=128))
                    h2T = hact.tile([128, HC * 1024], F32, tag="h2T", name="h2T")

                    def wrh(t, c, pt):
                        nc.scalar.copy(h2T[:, c * 1024 + t * 128: c * 1024 + (t + 1) * 128], pt[:])

                    gather_transpose_T(hsc, wrh, h_ag[1], hidxF_sb, 8, HC,
                                       extra=(h_ag[1], hidxB_sb, HL))
                    dense_b_c = load_cols(hcon, dense_b, ZC, "c_db")
                    normg_c = load_cols(hcon, normg, ZC, "c_ng")
                    normb_c = load_cols(hcon, normb, ZC, "c_nb")
                    pool_b1_c = load_cols(hcon, pool_b1, ZC, "c_pb1")
                    z0 = hact.tile([128, ZC * 1024], F32, tag="z0", name="z0")
                    proj_T(z0, h2T, dense_w, HC, ZC, 1024, bias_col=dense_b_c, func=AF.Relu)
                    zT = hact.tile([128, ZC * 1024], F32, tag="zT", name="zT")
                    ln_T(hsc, zT, z0, ZC, 1024, normg_c, normb_c, HL)
                    clsw_sb = hcon.tile([128, ZC * NUM_LABELS], F32)
                    for k in range(ZC):
                        dma(clsw_sb[:, k * NUM_LABELS:(k + 1) * NUM_LABELS], cls_w[k * 128:(k + 1) * 128, :])
                    clsb_b = hcon.tile([128, NUM_LABELS], F32)
                    dma(clsb_b[:], cls_b[:].to_broadcast([128, NUM_LABELS]))
                    for t in range(8):
                        pe = psp.tile([128, NUM_LABELS], F32, tag="A", bufs=2, name="pe")
                        for k in range(ZC):
                            nc.tensor.matmul(out=pe[:],
                                             lhsT=zT[:, k * 1024 + t * 128: k * 1024 + (t + 1) * 128],
                                             rhs=clsw_sb[:, k * NUM_LABELS:(k + 1) * NUM_LABELS],
                                             start=(k == 0), stop=(k == ZC - 1))
                        esb = hsc.tile([128, NUM_LABELS], F32, tag="esb", name="esb")
                        nc.vector.tensor_add(esb[:], pe[:], clsb_b[:])
                        dma(emis_d[t * 128:(t + 1) * 128, :], esb[:])
                        dma(out_emis[t * 128:(t + 1) * 128, :], esb[:])
                    znat = hact.tile([128, 8 * HL], F32, tag="znat", name="znat")
                    for t in range(8):
                        for c in range(ZC):
                            pt = psp.tile([128, 128], F32, tag="A", bufs=2, name="pt")
                            nc.tensor.transpose(out=pt[:], in_=zT[:, c * 1024 + t * 128: c * 1024 + (t + 1) * 128],
                                                identity=ident[:])
                            nc.scalar.copy(znat[:, t * HL + c * 128: t * HL + (c + 1) * 128], pt[:])
                    sentT = hsc.tile([128, ZC * 4], F32, tag="sentT", name="sentT")
                    for s in range(4):
                        wrow = rows.tile([1, 256], F32, tag="wrow", name="wrow")
                        pw = psp.tile([1, 256], F32, tag="B", bufs=6, name="pw")
                        for qc in range(2):
                            pS = psp.tile([128, 256], F32, tag="B", bufs=6, name="pSz")
                            for k in range(ZC):
                                nc.tensor.matmul(
                                    out=pS[:],
                                    lhsT=zT[:, k * 1024 + s * 256 + qc * 128: k * 1024 + s * 256 + (qc + 1) * 128],
                                    rhs=zT[:, k * 1024 + s * 256: k * 1024 + (s + 1) * 256],
                                    start=(k == 0), stop=(k == ZC - 1))
                            nm = hsc.tile([128, 1], F32, tag="atts", name="nmz")
                            nc.vector.reduce_max(nm[:], pS[:], axis=AX, negate=True)
                            ex = hsc.tile([128, 256], F32, tag="attx", name="exz")
                            sume = hsc.tile([128, 1], F32, tag="atts", name="sumez")
                            nc.scalar.activation(ex[:], pS[:], AF.Exp, bias=nm[:, 0:1],
                                                 scale=1.0, accum_out=sume[:, 0:1])
                            rs = hsc.tile([128, 1], F32, tag="atts", name="rsz")
                            nc.vector.reciprocal(rs[:], sume[:])
                            nc.vector.tensor_scalar(ex[:], ex[:], rs[:, 0:1], None, op0=ALU.mult)
                            nc.tensor.matmul(out=pw[:], lhsT=ones_col[:], rhs=ex[:],
                                             start=(qc == 0), stop=(qc == 1))
                            if qc == 1:
                                nc.scalar.copy(wrow[:], pw[:])
                        wcol = hsc.tile([128, 2], F32, tag="wcol", name="wcol")
                        for kt in range(2):
                            pt = psp.tile([128, 128], F32, tag="A", bufs=2, name="ptw")
                            nc.tensor.transpose(out=pt[:, 0:1], in_=wrow[:, kt * 128:(kt + 1) * 128],
                                                identity=ident[:1, :1])
                            nc.scalar.copy(wcol[:, kt:kt + 1], pt[:, 0:1])
                        for n in range(ZC):
                            psn = psp.tile([128, 1], F32, tag="A", bufs=2, name="psn")
                            for kt in range(2):
                                nc.tensor.matmul(
                                    out=psn[:],
                                    lhsT=znat[:, (2 * s + kt) * HL + n * 128: (2 * s + kt) * HL + (n + 1) * 128],
                                    rhs=wcol[:, kt:kt + 1],
                                    start=(kt == 0), stop=(kt == 1))
                            nc.scalar.mul(sentT[:, n * 4 + s: n * 4 + s + 1], psn[:], 1.0 / 256.0)
                    y1p = hsc.tile([128, ZC * 4], F32, tag="y1p", name="y1p")
                    for n in range(ZC):
                        p = psp.tile([128, 4], F32, tag="A", bufs=2, name="pl1")
                        for k in range(ZC):
                            w = wt.tile([128, 128], F32, tag="w", name="pw1")
                            dma(w[:], pool_w1[k * 128:(k + 1) * 128, n * 128:(n + 1) * 128])
                            nc.tensor.matmul(out=p[:], lhsT=w[:], rhs=sentT[:, k * 4:(k + 1) * 4],
                                             start=(k == 0), stop=(k == ZC - 1))
                        nc.scalar.activation(y1p[:, n * 4:(n + 1) * 4], p[:], AF.Relu,
                                             bias=pool_b1_c[:, n:n + 1], scale=1.0)
                    w2sb = hcon.tile([128, ZC * 2], F32)
                    for k in range(ZC):
                        dma(w2sb[:, k * 2:(k + 1) * 2], pool_w2[k * 128:(k + 1) * 128, :])
                    plg = psp.tile([2, 4], F32, tag="A", bufs=2, name="plg")
                    for k in range(ZC):
                        nc.tensor.matmul(out=plg[:], lhsT=w2sb[:, k * 2:(k + 1) * 2],
                                         rhs=y1p[:, k * 4:(k + 1) * 4],
                                         start=(k == 0), stop=(k == ZC - 1))
                    pb2 = hcon.tile([2, 1], F32)
                    dma(pb2[:], pool_b2[:])
                    lgT = hsc.tile([2, 4], F32, tag="lgT", name="lgT")
                    nc.scalar.activation(lgT[:], plg[:], AF.Identity, bias=pb2[:, 0:1], scale=1.0)
                    dma(slg_d[:].rearrange("s p -> p s"), lgT[:])
                    dma(out_slg[:].rearrange("s p -> p s"), lgT[:])
                    nc.gpsimd.collective_compute("AllGather", ALU.bypass, replica_groups=GRP,
                                                 ins=[slg_d[:]], outs=[slg_ag[:]])
                    nc.gpsimd.collective_compute("AllGather", ALU.bypass, replica_groups=GRP,
                                                 ins=[emis_d[:]], outs=[emis_ag[:]])

                    # ---------------- full-batch CRF ----------------
                    B = BATCH
                    emis = hcon.tile([B, SEQ * 5], F32)
                    dma(emis[:], emis_ag[:].rearrange("(b t) j -> b (t j)", b=B))
                    trans_b = hcon.tile([B, 25], F32)
                    dma(trans_b[:], crf_trans[:].to_broadcast([B, 25]))
                    start_b = hcon.tile([B, 5], F32)
                    dma(start_b[:], crf_start[:].to_broadcast([B, 5]))
                    end_b = hcon.tile([B, 5], F32)
                    dma(end_b[:], crf_end[:].to_broadcast([B, 5]))
                    iota5_b = hcon.tile([B, 5], F32)
                    dma(iota5_b[:], iota5[:].to_broadcast([B, 5]))
                    iota5m8_b = hcon.tile([B, 5], F32)
                    dma(iota5m8_b[:], iota5m8[:].to_broadcast([B, 5]))
                    iota2_b = hcon.tile([B, 2], F32)
                    dma(iota2_b[:], iota2[:].to_broadcast([B, 2]))
                    lab_i = hcon.tile([B, SEQ], I32)
                    dma(lab_i[:], labels_full[:])
                    lab_f = hcon.tile([B, SEQ], F32)
                    nc.vector.tensor_copy(lab_f[:], lab_i[:])
                    mask_i = hcon.tile([B, SEQ], I32)
                    dma(mask_i[:], mask_full[:])
                    mask_f = hcon.tile([B, SEQ], F32)
                    nc.vector.tensor_copy(mask_f[:], mask_i[:])
                    slab_i = hcon.tile([B, 1], I32)
                    dma(slab_i[:], slab_full[:])
                    slab_f = hcon.tile([B, 1], F32)
                    nc.vector.tensor_copy(slab_f[:], slab_i[:])

                    trans_ij = trans_b[:].rearrange("b (i j) -> b i j", j=5)

                    def bcast_ij(ap):
                        return ap.rearrange("b (i o) -> b i o", o=1).to_broadcast([B, 5, 5])

                    # ---- logZ forward scan ----
                    score = hsc.tile([B, 5], F32, tag="lzsc", name="score0")
                    nc.vector.tensor_add(score[:], start_b[:], emis[:, 0:5])
                    for t in range(1, SEQ):
                        cand = hsc.tile([B, 25], F32, tag="cand", name="cand")
                        nc.vector.tensor_tensor(cand[:].rearrange("b (i j) -> b i j", j=5),
                                                bcast_ij(score[:]), trans_ij, op=ALU.add)
                        nm = hsc.tile([B, 1], F32, tag="lzs1", name="nmt")
                        nc.vector.reduce_max(nm[:], cand[:], axis=AX, negate=True)
                        ex = hsc.tile([B, 25], F32, tag="cexp", name="ext")
                        nc.scalar.activation(ex[:], cand[:], AF.Exp, bias=nm[:, 0:1], scale=1.0)
                        sj = hsc.tile([B, 5], F32, tag="sj", name="sj")
                        nc.vector.tensor_add(sj[:], ex[:, 0:5], ex[:, 5:10])
                        nc.vector.tensor_add(sj[:], sj[:], ex[:, 10:15])
                        nc.vector.tensor_add(sj[:], sj[:], ex[:, 15:20])
                        nc.vector.tensor_add(sj[:], sj[:], ex[:, 20:25])
                        lg = hsc.tile([B, 5], F32, tag="lgg", name="lg")
                        nc.scalar.activation(lg[:], sj[:], AF.Ln)
                        score2 = hsc.tile([B, 5], F32, tag="lzsc", name="score")
                        nc.vector.scalar_tensor_tensor(score2[:], lg[:], nm[:, 0:1],
                                                       emis[:, t * 5:(t + 1) * 5],
                                                       op0=ALU.subtract, op1=ALU.add)
                        score = score2
                    fin = hsc.tile([B, 5], F32, tag="fin", name="fin")
                    nc.vector.tensor_add(fin[:], score[:], end_b[:])
                    nmf = hsc.tile([B, 1], F32, tag="lzs1", name="nmf")
                    nc.vector.reduce_max(nmf[:], fin[:], axis=AX, negate=True)
                    exf = hsc.tile([B, 5], F32, tag="fin2", name="exf")
                    sef = hsc.tile([B, 1], F32, tag="lzs2", name="sef")
                    nc.scalar.activation(exf[:], fin[:], AF.Exp, bias=nmf[:, 0:1], scale=1.0,
                                         accum_out=sef[:, 0:1])
                    logz = hsc.tile([B, 1], F32, tag="logz", name="logz")
                    nc.scalar.activation(logz[:], sef[:], AF.Ln)
                    nc.vector.tensor_sub(logz[:], logz[:], nmf[:])

                    # ---- gold path score ----
                    oh = hcon.tile([B, SEQ * 5], F32)
                    oh3 = oh[:].rearrange("b (t j) -> b t j", j=5)
                    nc.vector.tensor_tensor(
                        oh3,
                        lab_f[:].rearrange("b (t o) -> b t o", o=1).to_broadcast([B, SEQ, 5]),
                        iota5_b[:].rearrange("b (o j) -> b o j", o=1).to_broadcast([B, SEQ, 5]),
                        op=ALU.is_equal)
                    esel = hsc.tile([B, SEQ], F32, tag="esel", name="esel")
                    prod = hcon.tile([B, SEQ * 5], F32)
                    nc.vector.tensor_mul(prod[:], oh[:], emis[:])
                    nc.vector.reduce_sum(esel[:].rearrange("b (t o) -> b t o", o=1),
                                         prod[:].rearrange("b (t j) -> b t j", j=5), axis=AX)
                    trtmp = hcon.tile([B, (SEQ - 1) * 5], F32)
                    nc.vector.memset(trtmp[:], 0.0)
                    tmp_i = hsc.tile([B, (SEQ - 1) * 5], F32, tag="tmpi", name="tmpi")
                    for i in range(5):
                        nc.vector.tensor_tensor(
                            tmp_i[:].rearrange("b (t j) -> b t j", j=5),
                            oh3[:, 0:SEQ - 1, i:i + 1].to_broadcast([B, SEQ - 1, 5]),
                            trans_b[:, i * 5:(i + 1) * 5].rearrange("b (o j) -> b o j", o=1).to_broadcast([B, SEQ - 1, 5]),
                            op=ALU.mult)
                        nc.vector.tensor_add(trtmp[:], trtmp[:], tmp_i[:])
                    trsel = hsc.tile([B, SEQ - 1], F32, tag="trsel", name="trsel")
                    nc.vector.tensor_mul(tmp_i[:], trtmp[:], oh[:, 5:])
                    nc.vector.reduce_sum(trsel[:].rearrange("b (t o) -> b t o", o=1),
                                         tmp_i[:].rearrange("b (t j) -> b t j", j=5), axis=AX)
                    st0 = hsc.tile([B, 1], F32, tag="st0", name="st0")
                    t5 = hsc.tile([B, 5], F32, tag="t5", name="t5")
                    nc.vector.tensor_mul(t5[:], oh[:, 0:5], start_b[:])
                    nc.vector.reduce_sum(st0[:], t5[:], axis=AX)
                    t5b = hsc.tile([B, 5], F32, tag="t5", name="t5b")
                    nc.vector.tensor_mul(t5b[:], oh[:, (SEQ - 1) * 5: SEQ * 5], end_b[:])
                    endt = hsc.tile([B, 1], F32, tag="endt", name="endt")
                    nc.vector.reduce_sum(endt[:], t5b[:], axis=AX)
                    tre = hsc.tile([B, SEQ - 1], F32, tag="tre", name="tre")
                    nc.vector.tensor_add(tre[:], trsel[:], esel[:, 1:])
                    nc.vector.tensor_mul(tre[:], tre[:], mask_f[:, 1:])
                    smid = hsc.tile([B, 1], F32, tag="smid", name="smid")
                    nc.vector.reduce_sum(smid[:], tre[:], axis=AX)
                    gold = hsc.tile([B, 1], F32, tag="gold", name="gold")
                    nc.vector.tensor_add(gold[:], st0[:], smid[:])
                    nc.vector.tensor_add(gold[:], gold[:], endt[:])
                    nc.vector.tensor_add(gold[:], gold[:], esel[:, 0:1])
                    llh = hsc.tile([B, 1], F32, tag="llh", name="llh")
                    nc.vector.tensor_sub(llh[:], gold[:], logz[:])

                    # ---- viterbi ----
                    vsc = hsc.tile([B, 5], F32, tag="vsc", name="vsc0")
                    nc.vector.tensor_add(vsc[:], start_b[:], emis[:, 0:5])
                    hist = hcon.tile([B, (SEQ - 1) * 5], F32)
                    for t in range(1, SEQ):
                        cand = hsc.tile([B, 25], F32, tag="cand", name="vcand")
                        nc.vector.tensor_tensor(cand[:].rearrange("b (i j) -> b i j", j=5),
                                                bcast_ij(vsc[:]), trans_ij, op=ALU.add)
                        m5 = hsc.tile([B, 5], F32, tag="m5", name="m5")
                        nc.vector.tensor_max(m5[:], cand[:, 0:5], cand[:, 5:10])
                        nc.vector.tensor_max(m5[:], m5[:], cand[:, 10:15])
                        nc.vector.tensor_max(m5[:], m5[:], cand[:, 15:20])
                        nc.vector.tensor_max(m5[:], m5[:], cand[:, 20:25])
                        idxe = hsc.tile([B, 5], F32, tag="idxe", name="idxe")
                        eq = hsc.tile([B, 5], F32, tag="eq", name="eq0")
                        nc.vector.tensor_tensor(eq[:], cand[:, 0:5], m5[:], op=ALU.is_equal)
                        nc.vector.tensor_scalar(idxe[:], eq[:], -8.0, None, op0=ALU.mult)
                        for i in range(1, 5):
                            eqi = hsc.tile([B, 5], F32, tag="eq", name="eqi")
                            nc.vector.tensor_tensor(eqi[:], cand[:, i * 5:(i + 1) * 5], m5[:], op=ALU.is_equal)
                            nc.vector.scalar_tensor_tensor(idxe[:], eqi[:], float(i - 8), idxe[:],
                                                           op0=ALU.mult, op1=ALU.min)
                        nc.vector.tensor_scalar(hist[:, (t - 1) * 5: t * 5], idxe[:], 8.0, None, op0=ALU.add)
                        vsc2 = hsc.tile([B, 5], F32, tag="vsc", name="vsc")
                        nc.vector.tensor_add(vsc2[:], m5[:], emis[:, t * 5:(t + 1) * 5])
                        vsc = vsc2
                    fine = hsc.tile([B, 5], F32, tag="fine", name="fine")
                    nc.vector.tensor_add(fine[:], vsc[:], end_b[:])
                    mfin = hsc.tile([B, 1], F32, tag="mfin", name="mfin")
                    nc.vector.reduce_max(mfin[:], fine[:], axis=AX)
                    eqf = hsc.tile([B, 5], F32, tag="eqf", name="eqf")
                    nc.vector.tensor_scalar(eqf[:], fine[:], mfin[:, 0:1], None, op0=ALU.is_equal)
                    ence = hsc.tile([B, 5], F32, tag="ence", name="ence")
                    nc.vector.tensor_mul(ence[:], eqf[:], iota5m8_b[:])
                    cur = hsc.tile([B, 1], F32, tag="cur", name="cur0")
                    nc.vector.tensor_reduce(cur[:], ence[:], axis=AX, op=ALU.min)
                    nc.vector.tensor_scalar(cur[:], cur[:], 8.0, None, op0=ALU.add)
                    preds = hcon.tile([B, SEQ], F32)
                    nc.vector.tensor_copy(preds[:, SEQ - 1: SEQ], cur[:])
                    for t in range(SEQ - 2, -1, -1):
                        ohc = hsc.tile([B, 5], F32, tag="ohc", name="ohc")
                        nc.vector.tensor_scalar(ohc[:], iota5_b[:], cur[:, 0:1], None, op0=ALU.is_equal)
                        nc.vector.tensor_mul(ohc[:], ohc[:], hist[:, t * 5:(t + 1) * 5])
                        cur = hsc.tile([B, 1], F32, tag="cur", name="cur")
                        nc.vector.reduce_sum(cur[:], ohc[:], axis=AX)
                        nc.vector.tensor_copy(preds[:, t:t + 1], cur[:])
                    dma(out_preds[:], preds[:])

                    # ---- sentence CE + loss ----
                    slg_sb = hcon.tile([B, 2], F32)
                    dma(slg_sb[:], slg_ag[:])
                    nm2 = hsc.tile([B, 1], F32, tag="nm2", name="nm2")
                    nc.vector.reduce_max(nm2[:], slg_sb[:], axis=AX, negate=True)
                    ex2 = hsc.tile([B, 2], F32, tag="ex2", name="ex2")
                    se2 = hsc.tile([B, 1], F32, tag="se2", name="se2")
                    nc.scalar.activation(ex2[:], slg_sb[:], AF.Exp, bias=nm2[:, 0:1], scale=1.0,
                                         accum_out=se2[:, 0:1])
                    lse2 = hsc.tile([B, 1], F32, tag="lse2", name="lse2")
                    nc.scalar.activation(lse2[:], se2[:], AF.Ln)
                    nc.vector.tensor_sub(lse2[:], lse2[:], nm2[:])
                    sel2 = hsc.tile([B, 2], F32, tag="sel2", name="sel2")
                    nc.vector.tensor_scalar(sel2[:], iota2_b[:], slab_f[:, 0:1], None, op0=ALU.is_equal)
                    nc.vector.tensor_mul(sel2[:], sel2[:], slg_sb[:])
                    pick = hsc.tile([B, 1], F32, tag="pick", name="pick")
                    nc.vector.reduce_sum(pick[:], sel2[:], axis=AX)
                    ce = hsc.tile([B, 1], F32, tag="ce", name="ce")
                    nc.vector.tensor_sub(ce[:], lse2[:], pick[:])
                    psl = psp.tile([1, 1], F32, tag="A", bufs=2, name="psl")
                    nc.tensor.matmul(out=psl[:], lhsT=ones_col[0:B, :], rhs=llh[:], start=True, stop=True)
                    sllh = hsc.tile([1, 1], F32, tag="sllh", name="sllh")
                    nc.scalar.copy(sllh[:], psl[:])
                    psc_ = psp.tile([1, 1], F32, tag="A", bufs=2, name="psc_")
                    nc.tensor.matmul(out=psc_[:], lhsT=ones_col[0:B, :], rhs=ce[:], start=True, stop=True)
                    sce = hsc.tile([1, 1], F32, tag="sce", name="sce")
                    nc.scalar.copy(sce[:], psc_[:])
                    lossa = hsc.tile([1, 1], F32, tag="lossa", name="lossa")
                    nc.scalar.mul(lossa[:], sllh[:], -0.7 / BATCH)
                    lossb = hsc.tile([1, 1], F32, tag="lossb", name="lossb")
                    nc.scalar.mul(lossb[:], sce[:], 0.3 / BATCH)
                    loss = hsc.tile([1, 1], F32, tag="loss", name="loss")
                    nc.vector.tensor_add(loss[:], lossa[:], lossb[:])
                    dma(out_loss[:], loss[:])

    nc.compile()
    return nc


# =====================================================================
# Host side
# =====================================================================
def _col(v, nch):
    return np.ascontiguousarray(np.asarray(v, np.float32).reshape(nch, 128).T)


def _prep_inmaps(input_ids, attention_mask, labels, sentence_labels, params, nl,
                 run_lstm, run_head):
    p = params
    f32 = lambda a: np.ascontiguousarray(np.asarray(a, np.float32))
    i32 = lambda a: np.ascontiguousarray(np.asarray(a, np.int32))

    common = {}
    common["wemb"] = f32(p["word_emb"])
    common["pos_type"] = f32(np.asarray(p["pos_emb"])[:SEQ] + np.asarray(p["type_emb"])[None, :])
    common["embg"] = f32(p["emb_ln_g"]).reshape(1, D)
    common["embb"] = f32(p["emb_ln_b"]).reshape(1, D)
    for l in range(nl):
        lp = p["layers"][l]
        common[f"L{l}_wq"] = f32(lp["wq"]); common[f"L{l}_wk"] = f32(lp["wk"])
        common[f"L{l}_wv"] = f32(lp["wv"]); common[f"L{l}_wo"] = f32(lp["wo"])
        common[f"L{l}_bqkv"] = np.concatenate(
            [_col(lp["bq"], DC), _col(lp["bk"], DC), _col(lp["bv"], DC)], axis=1)
        common[f"L{l}_bo"] = _col(lp["bo"], DC)
        common[f"L{l}_ln1g"] = _col(lp["ln1_g"], DC); common[f"L{l}_ln1b"] = _col(lp["ln1_b"], DC)
        common[f"L{l}_w1"] = f32(lp["w1"]); common[f"L{l}_b1"] = _col(lp["b1"], FC)
        common[f"L{l}_w2"] = f32(lp["w2"]); common[f"L{l}_b2"] = _col(lp["b2"], DC)
        common[f"L{l}_ln2g"] = _col(lp["ln2_g"], DC); common[f"L{l}_ln2b"] = _col(lp["ln2_b"], DC)
    if run_head:
        common["dense_w"] = f32(p["dense_w"])
        common["dense_b"] = _col(p["dense_b"], ZC)
        common["normg"] = _col(p["norm_g"], ZC); common["normb"] = _col(p["norm_b"], ZC)
        common["cls_w"] = f32(p["cls_w"]); common["cls_b"] = f32(p["cls_b"]).reshape(1, NUM_LABELS)
        common["pool_w1"] = f32(p["pool_w1"]); common["pool_b1"] = _col(p["pool_b1"], ZC)
        common["pool_w2"] = f32(p["pool_w2"])
        common["pool_b2"] = f32(p["pool_b2"]).reshape(2, 1)
        common["crf_start"] = f32(p["crf_start"]).reshape(1, 5)
        common["crf_end"] = f32(p["crf_end"]).reshape(1, 5)
        common["crf_trans"] = f32(p["crf_trans"]).reshape(1, 25)
        common["iota5"] = np.arange(5, dtype=np.float32).reshape(1, 5)
        common["iota5m8"] = (np.arange(5, dtype=np.float32) - 8.0).reshape(1, 5)
        common["iota2"] = np.arange(2, dtype=np.float32).reshape(1, 2)
        common["labels_full"] = i32(labels)
        common["mask_full"] = i32(attention_mask)
        common["slab_full"] = i32(sentence_labels).reshape(BATCH, 1)

    in_maps = []
    ids_np = np.asarray(input_ids, np.int32)
    for c in range(NCORES):
        m = dict(common)
        m["ids"] = ids_np[4 * c:4 * c + 4].reshape(TOK, 1).copy()
        if run_lstm:
            d, g = c // 4, c % 4
            for ll in range(2):
                lp = p["lstm"][ll]["fwd" if d == 0 else "bwd"]
                m[f"wih{ll}"] = np.ascontiguousarray(np.asarray(lp["wih"], np.float32).T)
                whhT = np.asarray(lp["whh"], np.float32).T
                m[f"whh{ll}"] = np.ascontiguousarray(
                    whhT.reshape(3, 128, GC, 128).transpose(1, 0, 2, 3).reshape(128, 3 * G4)
                ).astype(np.float16)
                m[f"lbc{ll}"] = _col(lp["b"], GC)
            tau = np.arange(SEQ)
            tnat = tau if d == 0 else (SEQ - 1 - tau)
            qv = 8 * g + np.arange(LB)
            l1 = (qv[None, :] // 4) * TOK + (qv[None, :] % 4) * SEQ + tnat[:, None]
            m["lidx1"] = i32(l1.reshape(LTOK, 1))
            l2F = (qv[None, :] // 8) * (LB * SEQ) + (qv[None, :] % 8) * SEQ + tnat[:, None]
            l2B = (4 + qv[None, :] // 8) * (LB * SEQ) + (qv[None, :] % 8) * SEQ + (SEQ - 1 - tnat)[:, None]
            m["lidx2F"] = i32(l2F.reshape(LTOK, 1))
            m["lidx2B"] = i32(l2B.reshape(LTOK, 1))
        if run_head:
            t = np.arange(SEQ)
            qv = 4 * c + np.arange(4)
            hF = (qv[:, None] // 8) * (LB * SEQ) + (qv[:, None] % 8) * SEQ + t[None, :]
            hB = (4 + qv[:, None] // 8) * (LB * SEQ) + (qv[:, None] % 8) * SEQ + (SEQ - 1 - t)[None, :]
            m["hidxF"] = i32(hF.reshape(TOK, 1))
            m["hidxB"] = i32(hB.reshape(TOK, 1))
        in_maps.append(m)
    return in_maps


def run_cores(input_ids, attention_mask, labels, sentence_labels, params,
              nl=NL, run_lstm=True, run_head=True, trace=False):
    from concourse.bass_utils import run_bass_kernel_spmd
    key = (nl, run_lstm, run_head)
    if key not in _CACHE:
        _CACHE[key] = _build(nl, run_lstm, run_head)
    nc = _CACHE[key]
    in_maps = _prep_inmaps(input_ids, attention_mask, labels, sentence_labels,
                           params, nl, run_lstm, run_head)
    return run_bass_kernel_spmd(nc, in_maps, list(range(NCORES)), trace=trace)


def kernel(input_ids, attention_mask, labels, sentence_labels, params):
    res = run_cores(input_ids, attention_mask, labels, sentence_labels, params)
    r = res.results
    loss = np.asarray(np.float32(r[0]["out_loss"][0, 0]))
    emissions = np.stack([r[c]["out_emis"].reshape(4, SEQ, NUM_LABELS) for c in range(NCORES)])
    emissions = np.ascontiguousarray(emissions.reshape(BATCH, SEQ, NUM_LABELS))
    preds = np.rint(r[0]["out_preds"]).astype(np.int32)
    sent_logits = np.concatenate([r[c]["out_slg"] for c in range(NCORES)], axis=0)
    return loss, emissions, preds, sent_logits


# revision 10
# speedup vs baseline: 1.5691x; 1.5691x over previous
"""Trainium2 Bass kernel for EnhancedBertForIdiomDetection (SPMD, 8 cores).

Sharding:
  - BERT encoder + head: data parallel, 4 seqs (1024 tokens) per core.
  - BiLSTM: 8 cores = 2 directions x 4 seq-groups (8 seqs per core); the
    backward direction's time reversal is handled by per-core gather index
    tensors so every core runs the identical SPMD program.
  - CRF logZ / gold score / viterbi: full batch (32 seqs on partitions),
    computed redundantly on every core after an AllGather of emissions.
BERT activations are kept transposed ([feature, token]) so the matmul chain
needs no activation transposes; LayerNorm over the feature (partition) dim
uses TensorE ones-reductions.
"""
import sys

sys.path.insert(0, "/opt/trn_rl_repo")

import numpy as np

V, SEQ, BATCH = 119547, 256, 32
D, NL, NH, DH, FF = 768, 12, 12, 64, 3072
HL, NUM_LABELS = 384, 5
NCORES = 8
TOK = 4 * SEQ
LTOK = 8 * SEQ
DC = D // 128
FC = FF // 128
HC = (2 * HL) // 128
ZC = HL // 128
G4 = 4 * HL
GC = G4 // 128
LB = 8

_CACHE = {}


def _build(nl, run_lstm, run_head):
    import concourse.bass as bass_mod
    import concourse.mybir as mybir
    import concourse.tile as tile
    from concourse import bacc
    from concourse.masks import make_identity

    F32 = mybir.dt.float32
    F16 = mybir.dt.float16
    I32 = mybir.dt.int32
    AF = mybir.ActivationFunctionType
    ALU = mybir.AluOpType
    AX = mybir.AxisListType.X
    IOff = bass_mod.IndirectOffsetOnAxis

    nc = bacc.Bacc(None)
    dp = lambda name, shape, dt=F32: nc.declare_dram_parameter(name, list(shape), dt, isOutput=False)
    do = lambda name, shape, dt=F32: nc.declare_dram_parameter(name, list(shape), dt, isOutput=True)

    ids = dp("ids", [TOK, 1], I32)
    wemb = dp("wemb", [V, D])
    pos_type = dp("pos_type", [SEQ, D])
    embg = dp("embg", [1, D])
    embb = dp("embb", [1, D])
    LW = []
    for l in range(nl):
        LW.append(dict(
            wq=dp(f"L{l}_wq", [D, D]), wk=dp(f"L{l}_wk", [D, D]),
            wv=dp(f"L{l}_wv", [D, D]), wo=dp(f"L{l}_wo", [D, D]),
            bqkv=dp(f"L{l}_bqkv", [128, 3 * DC]), bo=dp(f"L{l}_bo", [128, DC]),
            ln1g=dp(f"L{l}_ln1g", [128, DC]), ln1b=dp(f"L{l}_ln1b", [128, DC]),
            w1=dp(f"L{l}_w1", [D, FF]), b1=dp(f"L{l}_b1", [128, FC]),
            w2=dp(f"L{l}_w2", [FF, D]), b2=dp(f"L{l}_b2", [128, DC]),
            ln2g=dp(f"L{l}_ln2g", [128, DC]), ln2b=dp(f"L{l}_ln2b", [128, DC]),
        ))
    if run_lstm:
        lidx1 = dp("lidx1", [LTOK, 1], I32)
        lidx2F = dp("lidx2F", [LTOK, 1], I32)
        lidx2B = dp("lidx2B", [LTOK, 1], I32)
        wihs = [dp(f"wih{ll}", [2 * HL, G4]) for ll in range(2)]
        whhp = [dp(f"whh{ll}", [128, 3 * G4], F16) for ll in range(2)]
        lbc = [dp(f"lbc{ll}", [128, GC]) for ll in range(2)]
    if run_head:
        hidxF = dp("hidxF", [TOK, 1], I32)
        hidxB = dp("hidxB", [TOK, 1], I32)
        dense_w = dp("dense_w", [2 * HL, HL])
        dense_b = dp("dense_b", [128, ZC])
        normg = dp("normg", [128, ZC])
        normb = dp("normb", [128, ZC])
        cls_w = dp("cls_w", [HL, NUM_LABELS])
        cls_b = dp("cls_b", [1, NUM_LABELS])
        pool_w1 = dp("pool_w1", [HL, HL])
        pool_b1 = dp("pool_b1", [128, ZC])
        pool_w2 = dp("pool_w2", [HL, 2])
        pool_b2 = dp("pool_b2", [2, 1])
        crf_start = dp("crf_start", [1, 5])
        crf_end = dp("crf_end", [1, 5])
        crf_trans = dp("crf_trans", [1, 25])
        iota5 = dp("iota5", [1, 5])
        iota5m8 = dp("iota5m8", [1, 5])
        iota2 = dp("iota2", [1, 2])
        labels_full = dp("labels_full", [BATCH, SEQ], I32)
        mask_full = dp("mask_full", [BATCH, SEQ], I32)
        slab_full = dp("slab_full", [BATCH, 1], I32)

    out_x = do("out_x", [TOK, D])
    if run_head:
        out_emis = do("out_emis", [TOK, NUM_LABELS])
        out_slg = do("out_slg", [4, 2])
        out_preds = do("out_preds", [BATCH, SEQ])
        out_loss = do("out_loss", [1, 1])

    xout_d = nc.dram_tensor("xout_d", [TOK, D], F32)
    if run_lstm:
        xag = nc.dram_tensor("xag", [NCORES * TOK, D], F32, addr_space="Shared")
        x2T_d = nc.dram_tensor("x2T_d", [128, DC * LTOK], F32)
        pre_d = nc.dram_tensor("pre_d", [128, SEQ * GC * LB], F32)
        h_loc = [nc.dram_tensor(f"h{ll}loc", [LB * SEQ, HL], F32) for ll in range(2)]
        h_ag = [nc.dram_tensor(f"h{ll}ag", [NCORES * LB * SEQ, HL], F32, addr_space="Shared")
                for ll in range(2)]
    if run_head:
        emis_d = nc.dram_tensor("emis_d", [TOK, NUM_LABELS], F32)
        emis_ag = nc.dram_tensor("emis_ag", [NCORES * TOK, NUM_LABELS], F32, addr_space="Shared")
        slg_d = nc.dram_tensor("slg_d", [4, 2], F32)
        slg_ag = nc.dram_tensor("slg_ag", [BATCH, 2], F32, addr_space="Shared")

    GRP = [list(range(NCORES))]

    with tile.TileContext(nc) as tc:
        import contextlib
        with contextlib.ExitStack() as ctx:
            gp = ctx.enter_context(tc.tile_pool(name="gp", bufs=1))
            rows = ctx.enter_context(tc.tile_pool(name="rows", bufs=3))
            wt = ctx.enter_context(tc.tile_pool(name="wt", bufs=8))
            psp = ctx.enter_context(tc.tile_pool(name="psp", bufs=1, space="PSUM"))

            ident = gp.tile([128, 128], F32)
            make_identity(nc, ident[:])
            ones_col = gp.tile([128, 1], F32)
            nc.vector.memset(ones_col[:], 1.0)
            eps_col = gp.tile([128, 1], F32)
            nc.vector.memset(eps_col[:], 1e-12)

            def dma(out, in_):
                nc.sync.dma_start(out=out, in_=in_)

            def load_cols(pool, dram_t, ncols, tag):
                t = pool.tile([128, ncols], F32, tag=tag, name=tag)
                dma(t[:], dram_t[:, 0:ncols])
                return t

            def ln_T(sc, dst, src, nch, ntok, g_col, b_col, dim):
                half = ntok // 2
                sums = rows.tile([1, ntok], F32, tag="lnrow", name="sums")
                sums2 = rows.tile([1, ntok], F32, tag="lnrow", name="sums2")
                for th in range(2):
                    p = psp.tile([1, half], F32, tag="A", bufs=2, name="lnp")
                    for c in range(nch):
                        nc.tensor.matmul(out=p[:], lhsT=ones_col[:],
                                         rhs=src[:, c * ntok + th * half: c * ntok + (th + 1) * half],
                                         start=(c == 0), stop=(c == nch - 1))
                    nc.scalar.copy(sums[:, th * half:(th + 1) * half], p[:])
                for th in range(2):
                    p = psp.tile([1, half], F32, tag="A", bufs=2, name="lnp2")
                    for c in range(nch):
                        sq = sc.tile([128, half], F32, tag="lnsq", name="sq")
                        nc.scalar.square(sq[:], src[:, c * ntok + th * half: c * ntok + (th + 1) * half])
                        nc.tensor.matmul(out=p[:], lhsT=ones_col[:], rhs=sq[:],
                                         start=(c == 0), stop=(c == nch - 1))
                    nc.scalar.copy(sums2[:, th * half:(th + 1) * half], p[:])
                mu = rows.tile([1, ntok], F32, tag="lnrow", name="mu")
                nc.vector.tensor_scalar(mu[:], sums[:], 1.0 / dim, None, op0=ALU.mult)
                var = rows.tile([1, ntok], F32, tag="lnrow", name="var")
                nc.vector.tensor_tensor(var[:], mu[:], mu[:], op=ALU.mult)
                nc.vector.scalar_tensor_tensor(var[:], sums2[:], 1.0 / dim, var[:],
                                               op0=ALU.mult, op1=ALU.subtract)
                srt = rows.tile([1, ntok], F32, tag="lnrow", name="srt")
                nc.scalar.activation(srt[:], var[:], AF.Sqrt, bias=eps_col[0:1, 0:1], scale=1.0)
                rstd = rows.tile([1, ntok], F32, tag="lnrow", name="rstd")
                nc.vector.reciprocal(rstd[:], srt[:])
                mub = sc.tile([128, ntok], F32, tag="lnbc", name="mub")
                rstdb = sc.tile([128, ntok], F32, tag="lnbc", name="rstdb")
                nc.gpsimd.partition_broadcast(mub[:], mu[:])
                nc.gpsimd.partition_broadcast(rstdb[:], rstd[:])
                for c in range(nch):
                    t1 = sc.tile([128, ntok], F32, tag="lnt1", name="t1")
                    nc.vector.tensor_sub(t1[:], src[:, c * ntok:(c + 1) * ntok], mub[:])
                    nc.vector.tensor_mul(t1[:], t1[:], rstdb[:])
                    nc.scalar.activation(dst[:, c * ntok:(c + 1) * ntok], t1[:],
                                         AF.Identity, bias=b_col[:, c:c + 1],
                                         scale=g_col[:, c:c + 1])

            def proj_T(dst, src, w_dram, nin_ch, nout_ch, ntok, bias_col=None,
                       func=None, badd=None):
                func = func if func is not None else AF.Identity
                half = 512
                wd3 = w_dram[:].rearrange("(k p) n -> p k n", p=128)
                for n in range(nout_ch):
                    ws = wt.tile([128, nin_ch * 128], F32, tag="wstrip", bufs=4, name="ws")
                    dma(ws[:], wd3[:, :, n * 128:(n + 1) * 128])
                    for th in range(ntok // half):
                        p = psp.tile([128, half], F32, tag="A", bufs=2, name="pp")
                        for k in range(nin_ch):
                            nc.tensor.matmul(out=p[:], lhsT=ws[:, k * 128:(k + 1) * 128],
                                             rhs=src[:, k * ntok + th * half: k * ntok + th * half + half],
                                             start=(k == 0), stop=(k == nin_ch - 1))
                        dsl = dst[:, n * ntok + th * half: n * ntok + th * half + half]
                        if badd is not None:
                            nc.vector.scalar_tensor_tensor(
                                dsl, p[:], bias_col[:, n:n + 1],
                                badd[:, n * ntok + th * half: n * ntok + th * half + half],
                                op0=ALU.add, op1=ALU.add)
                        else:
                            nc.scalar.activation(dsl, p[:], func,
                                                 bias=0.0 if bias_col is None else bias_col[:, n:n + 1],
                                                 scale=1.0)

            def gather_transpose_T(sc, dst_writer, src_dram, idx_sb, ntiles, nch, extra=None):
                for t in range(ntiles):
                    g = sc.tile([128, nch * 128], F32, tag="gath", name="g")
                    if extra is None:
                        nc.gpsimd.indirect_dma_start(out=g[:], out_offset=None, in_=src_dram[:],
                                                     in_offset=IOff(ap=idx_sb[:, t:t + 1], axis=0))
                    else:
                        src2, idx2, hw = extra
                        nc.gpsimd.indirect_dma_start(out=g[:, 0:hw], out_offset=None, in_=src_dram[:],
                                                     in_offset=IOff(ap=idx_sb[:, t:t + 1], axis=0))
                        nc.gpsimd.indirect_dma_start(out=g[:, hw:2 * hw], out_offset=None, in_=src2[:],
                                                     in_offset=IOff(ap=idx2[:, t:t + 1], axis=0))
                    for c in range(nch):
                        pt = psp.tile([128, 128], F32, tag="A", bufs=2, name="pt")
                        nc.tensor.transpose(out=pt[:], in_=g[:, c * 128:(c + 1) * 128], identity=ident[:])
                        dst_writer(t, c, pt)

            # ============================================================
            # Phase 1: embedding + BERT
            # ============================================================
            with tc.tile_pool(name="bact", bufs=1) as bact, \
                 tc.tile_pool(name="bsc", bufs=2) as bsc, \
                 tc.tile_pool(name="bcon", bufs=1) as bcon:
                ids_sb = bcon.tile([128, 8], I32)
                dma(ids_sb[:], ids[:].rearrange("(t p) o -> p (t o)", p=128))
                pos_sb = bcon.tile([128, 2 * D], F32)
                dma(pos_sb[:, 0:D], pos_type[0:128, :])
                dma(pos_sb[:, D:2 * D], pos_type[128:256, :])
                embg_b = bcon.tile([128, D], F32)
                embb_b = bcon.tile([128, D], F32)
                dma(embg_b[:], embg[:].to_broadcast([128, D]))
                dma(embb_b[:], embb[:].to_broadcast([128, D]))

                xT = bact.tile([128, DC * 1024], F32, tag="xT", name="xT")
                for t in range(8):
                    g = bsc.tile([128, D], F32, tag="emb", name="g")
                    nc.gpsimd.indirect_dma_start(out=g[:], out_offset=None, in_=wemb[:],
                                                 in_offset=IOff(ap=ids_sb[:, t:t + 1], axis=0))
                    nc.vector.tensor_add(g[:], g[:], pos_sb[:, (t % 2) * D:(t % 2 + 1) * D])
                    nmu = bsc.tile([128, 1], F32, tag="embs", name="nmu")
                    nc.vector.reduce_sum(nmu[:], g[:], axis=AX, negate=True)
                    nc.vector.tensor_scalar(nmu[:], nmu[:], 1.0 / D, None, op0=ALU.mult)
                    xc = bsc.tile([128, D], F32, tag="emb2", name="xc")
                    nc.vector.tensor_scalar(xc[:], g[:], nmu[:, 0:1], None, op0=ALU.add)
                    sqv = bsc.tile([128, D], F32, tag="emb3", name="sqv")
                    ssq = bsc.tile([128, 1], F32, tag="embs", name="ssq")
                    nc.scalar.activation(sqv[:], xc[:], AF.Square, accum_out=ssq[:, 0:1])
                    srt = bsc.tile([128, 1], F32, tag="embs", name="srt")
                    nc.scalar.activation(srt[:], ssq[:], AF.Sqrt, bias=eps_col[:, 0:1], scale=1.0 / D)
                    rstd = bsc.tile([128, 1], F32, tag="embs", name="rstd")
                    nc.vector.reciprocal(rstd[:], srt[:])
                    nc.vector.tensor_scalar(xc[:], xc[:], rstd[:, 0:1], None, op0=ALU.mult)
                    nc.vector.tensor_mul(xc[:], xc[:], embg_b[:])
                    nc.vector.tensor_add(xc[:], xc[:], embb_b[:])
                    for c in range(DC):
                        pt = psp.tile([128, 128], F32, tag="A", bufs=2, name="pt")
                        nc.tensor.transpose(out=pt[:], in_=xc[:, c * 128:(c + 1) * 128], identity=ident[:])
                        nc.scalar.copy(xT[:, c * 1024 + t * 128: c * 1024 + (t + 1) * 128], pt[:])

                for l in range(nl):
                    W = LW[l]
                    bqkv_c = load_cols(bcon, W["bqkv"], 3 * DC, "c_bqkv")
                    bo_c = load_cols(bcon, W["bo"], DC, "c_bo")
                    ln1g_c = load_cols(bcon, W["ln1g"], DC, "c_l1g")
                    ln1b_c = load_cols(bcon, W["ln1b"], DC, "c_l1b")
                    b1_c = load_cols(bcon, W["b1"], FC, "c_b1")
                    b2_c = load_cols(bcon, W["b2"], DC, "c_b2")
                    ln2g_c = load_cols(bcon, W["ln2g"], DC, "c_l2g")
                    ln2b_c = load_cols(bcon, W["ln2b"], DC, "c_l2b")
                    qT = bact.tile([128, DC * 1024], F32, tag="qT", name="qT")
                    kT = bact.tile([128, DC * 1024], F32, tag="kT", name="kT")
                    vT = bact.tile([128, DC * 1024], F32, tag="vT", name="vT")
                    for mi, (wm, tgt) in enumerate([(W["wq"], qT), (W["wk"], kT), (W["wv"], vT)]):
                        proj_T(tgt, xT, wm, DC, DC, 1024,
                               bias_col=bqkv_c[:, mi * DC:(mi + 1) * DC])
                    vnat = bact.tile([128, 8 * D], F32, tag="vnat", name="vnat")
                    for t in range(8):
                        for c in range(DC):
                            pt = psp.tile([128, 128], F32, tag="A", bufs=2, name="pt")
                            nc.tensor.transpose(out=pt[:], in_=vT[:, c * 1024 + t * 128: c * 1024 + (t + 1) * 128],
                                                identity=ident[:])
                            nc.scalar.copy(vnat[:, t * D + c * 128: t * D + (c + 1) * 128], pt[:])
                    ctxT = bact.tile([128, DC * 1024], F32, tag="vT", name="ctxT")
                    for s in range(4):
                        for h in range(NH):
                            hc, po = h // 2, (h % 2) * DH
                            pT = bsc.tile([128, 512], F32, tag="attp", name="pT")
                            for qc in range(2):
                                pS = psp.tile([128, 256], F32, tag="B", bufs=6, name="pS")
                                nc.tensor.matmul(
                                    out=pS[:],
                                    lhsT=qT[po:po + DH, hc * 1024 + s * 256 + qc * 128: hc * 1024 + s * 256 + (qc + 1) * 128],
                                    rhs=kT[po:po + DH, hc * 1024 + s * 256: hc * 1024 + (s + 1) * 256],
                                    start=True, stop=True)
                                nm = bsc.tile([128, 1], F32, tag="atts", name="nm")
                                nc.vector.reduce_max(nm[:], pS[:], axis=AX, negate=True)
                                nc.vector.tensor_scalar(nm[:], nm[:], 0.125, None, op0=ALU.mult)
                                ex = bsc.tile([128, 256], F32, tag="attx", name="ex")
                                sume = bsc.tile([128, 1], F32, tag="atts", name="sume")
                                nc.scalar.activation(ex[:], pS[:], AF.Exp, bias=nm[:, 0:1],
                                                     scale=0.125, accum_out=sume[:, 0:1])
                                rs = bsc.tile([128, 1], F32, tag="atts", name="rs")
                                nc.vector.reciprocal(rs[:], sume[:])
                                nc.vector.tensor_scalar(ex[:], ex[:], rs[:, 0:1], None, op0=ALU.mult)
                                for kc in range(2):
                                    pt = psp.tile([128, 128], F32, tag="A", bufs=2, name="pt")
                                    nc.tensor.transpose(out=pt[:], in_=ex[:, kc * 128:(kc + 1) * 128],
                                                        identity=ident[:])
                                    nc.scalar.copy(pT[:, kc * 256 + qc * 128: kc * 256 + (qc + 1) * 128], pt[:])
                            pc = psp.tile([64, 256], F32, tag="B", bufs=6, name="pc")
                            for kc in range(2):
                                nc.tensor.matmul(
                                    out=pc[:],
                                    lhsT=vnat[:, (2 * s + kc) * D + h * DH: (2 * s + kc) * D + (h + 1) * DH],
                                    rhs=pT[:, kc * 256:(kc + 1) * 256],
                                    start=(kc == 0), stop=(kc == 1))
                            nc.scalar.copy(ctxT[po:po + DH, hc * 1024 + s * 256: hc * 1024 + (s + 1) * 256], pc[:])
                    y1 = bact.tile([128, DC * 1024], F32, tag="qT", name="y1")
                    proj_T(y1, ctxT, W["wo"], DC, DC, 1024, bias_col=bo_c, badd=xT)
                    x1T = bact.tile([128, DC * 1024], F32, tag="kT", name="x1T")
                    ln_T(bsc, x1T, y1, DC, 1024, ln1g_c, ln1b_c, D)
                    y2 = bact.tile([128, DC * 1024], F32, tag="qT", name="y2")
                    for th in range(2):
                        pouts = []
                        for pi in range(DC):
                            po_t = psp.tile([128, 512], F32, tag="B", bufs=6, name=f"pout{pi}")
                            pouts.append(po_t)
                        w13 = W["w1"][:].rearrange("(k p) n -> p k n", p=128)
                        for hcc in range(FC):
                            w1s = wt.tile([128, DC * 128], F32, tag="wstrip", bufs=4, name="w1s")
                            dma(w1s[:], w13[:, :, hcc * 128:(hcc + 1) * 128])
                            ph = psp.tile([128, 512], F32, tag="A", bufs=2, name="ph")
                            for k in range(DC):
                                nc.tensor.matmul(out=ph[:], lhsT=w1s[:, k * 128:(k + 1) * 128],
                                                 rhs=x1T[:, k * 1024 + th * 512: k * 1024 + th * 512 + 512],
                                                 start=(k == 0), stop=(k == DC - 1))
                            hsb = bsc.tile([128, 512], F32, tag="ffa", name="hsb")
                            nc.scalar.activation(hsb[:], ph[:], AF.Gelu, bias=b1_c[:, hcc:hcc + 1], scale=1.0)
                            w2s = wt.tile([128, DC * 128], F32, tag="wstrip", bufs=4, name="w2s")
                            dma(w2s[:], W["w2"][hcc * 128:(hcc + 1) * 128, :])
                            for n in range(DC):
                                nc.tensor.matmul(out=pouts[n][:], lhsT=w2s[:, n * 128:(n + 1) * 128], rhs=hsb[:],
                                                 start=(hcc == 0), stop=(hcc == FC - 1))
                        for n in range(DC):
                            nc.vector.scalar_tensor_tensor(
                                y2[:, n * 1024 + th * 512: n * 1024 + th * 512 + 512],
                                pouts[n][:], b2_c[:, n:n + 1],
                                x1T[:, n * 1024 + th * 512: n * 1024 + th * 512 + 512],
                                op0=ALU.add, op1=ALU.add)
                    xT = bact.tile([128, DC * 1024], F32, tag="xT", name="xTn")
                    ln_T(bsc, xT, y2, DC, 1024, ln2g_c, ln2b_c, D)

                for t in range(8):
                    xo = bsc.tile([128, D], F32, tag="emb", name="xo")
                    for c in range(DC):
                        pt = psp.tile([128, 128], F32, tag="A", bufs=2, name="pt")
                        nc.tensor.transpose(out=pt[:], in_=xT[:, c * 1024 + t * 128: c * 1024 + (t + 1) * 128],
                                            identity=ident[:])
                        nc.scalar.copy(xo[:, c * 128:(c + 1) * 128], pt[:])
                    dma(xout_d[t * 128:(t + 1) * 128, :], xo[:])
                    dma(out_x[t * 128:(t + 1) * 128, :], xo[:])

            # ============================================================
            # Phase 2: BiLSTM
            # ============================================================
            if run_lstm:
                nc.gpsimd.collective_compute("AllGather", ALU.bypass, replica_groups=GRP,
                                             ins=[xout_d[:]], outs=[xag[:]])
                x2T3 = x2T_d[:].rearrange("p (c r) -> p c r", r=LTOK)
                pre3 = pre_d[:].rearrange("p (t q) -> p t q", q=GC * LB)

                with tc.tile_pool(name="lsc", bufs=2) as lsc, \
                     tc.tile_pool(name="lcon", bufs=1) as lcon:
                    def lstm_layer(ll, src1, src2, idxF_d, idxB_d, h_loc_d):
                        idxF_sb = lcon.tile([128, 16], I32, tag="idxF", name="idxF")
                        dma(idxF_sb[:], idxF_d[:].rearrange("(t p) o -> p (t o)", p=128))
                        idxB_sb = None
                        if idxB_d is not None:
                            idxB_sb = lcon.tile([128, 16], I32, tag="idxB", name="idxB")
                            dma(idxB_sb[:], idxB_d[:].rearrange("(t p) o -> p (t o)", p=128))

                        def wr(t, c, pt):
                            stg = lsc.tile([128, 128], F32, tag="ltstg", name="stg")
                            nc.scalar.copy(stg[:], pt[:])
                            dma(x2T3[:, c:c + 1, t * 128:(t + 1) * 128].rearrange("p c r -> p (c r)"),
                                stg[:])

                        gather_transpose_T(lsc, wr, src1, idxF_sb, 16, DC,
                                           extra=None if idxB_d is None else (src2, idxB_sb, HL))
                        lbc_c = load_cols(lcon, lbc[ll], GC, "c_lbc")
                        for tc4 in range(4):
                            x2s = lsc.tile([128, DC * 512], F32, tag="x2s", name="x2s")
                            dma(x2s[:], x2T3[:, :, tc4 * 512:(tc4 + 1) * 512])
                            wih3 = wihs[ll][:].rearrange("(k p) n -> p k n", p=128)
                            for n in range(GC):
                                wis = wt.tile([128, DC * 128], F32, tag="wstrip", bufs=4, name="wis")
                                dma(wis[:], wih3[:, :, n * 128:(n + 1) * 128])
                                p = psp.tile([128, 512], F32, tag="A", bufs=2, name="pp")
                                for k in range(DC):
                                    nc.tensor.matmul(out=p[:], lhsT=wis[:, k * 128:(k + 1) * 128],
                                                     rhs=x2s[:, k * 512:(k + 1) * 512],
                                                     start=(k == 0), stop=(k == DC - 1))
                                stg = lsc.tile([128, 512], F32, tag="prestg", name="pstg")
                                nc.scalar.activation(stg[:], p[:], AF.Identity,
                                                     bias=lbc_c[:, n:n + 1], scale=1.0)
                                dma(pre3[:, tc4 * 64:(tc4 + 1) * 64, n * LB:(n + 1) * LB],
                                    stg[:].rearrange("p (t s) -> p t s", s=LB))
                        whh_sb = lcon.tile([128, 3 * G4], F16, tag="whh", name="whh")
                        dma(whh_sb[:], whhp[ll][:])
                        h16 = lsc.tile([128, 3 * LB], F16, tag="h16", name="h16i")
                        cst = lsc.tile([128, 3 * LB], F32, tag="cst", name="csti")
                        nc.vector.memset(h16[:], 0.0)
                        nc.vector.memset(cst[:], 0.0)
                        BLK = 16
                        for blk in range(SEQ // BLK):
                            preb = lsc.tile([128, BLK * GC * LB], F32, tag="preblk", name="preb")
                            dma(preb[:], pre3[:, blk * BLK:(blk + 1) * BLK, :])
                            hist = lsc.tile([128, BLK * 3 * LB], F32, tag="hist", name="hist")
                            for tl in range(BLK):
                                pg = psp.tile([128, GC * LB], F32, tag="A", bufs=2, name="pg")
                                for n in range(GC):
                                    for k in range(3):
                                        nc.tensor.matmul(
                                            out=pg[:, n * LB:(n + 1) * LB],
                                            lhsT=whh_sb[:, (k * GC + n) * 128:(k * GC + n + 1) * 128],
                                            rhs=h16[:, k * LB:(k + 1) * LB],
                                            start=(k == 0), stop=(k == 2))
                                gsb = lsc.tile([128, GC * LB], F32, tag="gsb", name="gsb")
                                nc.vector.tensor_add(gsb[:], pg[:], preb[:, tl * GC * LB:(tl + 1) * GC * LB])
                                sif = lsc.tile([128, 6 * LB], F32, tag="sif", name="sif")
                                nc.scalar.activation(sif[:], gsb[:, 0:6 * LB], AF.Sigmoid)
                                tg = lsc.tile([128, 3 * LB], F32, tag="tg", name="tg")
                                nc.scalar.activation(tg[:], gsb[:, 6 * LB:9 * LB], AF.Tanh)
                                so = lsc.tile([128, 3 * LB], F32, tag="so", name="so")
                                nc.scalar.activation(so[:], gsb[:, 9 * LB:12 * LB], AF.Sigmoid)
                                c2 = lsc.tile([128, 3 * LB], F32, tag="cst", name="c2")
                                nc.vector.tensor_mul(c2[:], sif[:, 3 * LB:6 * LB], cst[:])
                                t2 = lsc.tile([128, 3 * LB], F32, tag="t2", name="t2")
                                nc.vector.tensor_mul(t2[:], sif[:, 0:3 * LB], tg[:])
                                nc.vector.tensor_add(c2[:], c2[:], t2[:])
                                cst = c2
                                tch = lsc.tile([128, 3 * LB], F32, tag="tch", name="tch")
                                nc.scalar.activation(tch[:], cst[:], AF.Tanh)
                                hsl = hist[:, tl * 3 * LB:(tl + 1) * 3 * LB]
                                nc.vector.tensor_mul(hsl, so[:], tch[:])
                                h16 = lsc.tile([128, 3 * LB], F16, tag="h16", name="h16")
                                nc.vector.tensor_copy(h16[:], hsl)
                            hist3 = hist[:].rearrange("p (t q) -> p t q", q=3 * LB)
                            for c in range(3):
                                for s in range(LB):
                                    dma(h_loc_d[s * SEQ + blk * BLK: s * SEQ + (blk + 1) * BLK,
                                                c * 128:(c + 1) * 128].rearrange("t (f o) -> f t o", o=1),
                                        hist3[:, :, c * LB + s: c * LB + s + 1])

                    lstm_layer(0, xag, None, lidx1, None, h_loc[0])
                    nc.gpsimd.collective_compute("AllGather", ALU.bypass, replica_groups=GRP,
                                                 ins=[h_loc[0][:]], outs=[h_ag[0][:]])
                    lstm_layer(1, h_ag[0], h_ag[0], lidx2F, lidx2B, h_loc[1])
                    nc.gpsimd.collective_compute("AllGather", ALU.bypass, replica_groups=GRP,
                                                 ins=[h_loc[1][:]], outs=[h_ag[1][:]])

            # ============================================================
            # Phase 3: head + full-batch CRF
            # ============================================================
            if run_head:
                with tc.tile_pool(name="hact", bufs=1) as hact, \
                     tc.tile_pool(name="hsc", bufs=2) as hsc, \
                     tc.tile_pool(name="hcon", bufs=1) as hcon:
                    hidxF_sb = hcon.tile([128, 8], I32)
                    dma(hidxF_sb[:], hidxF[:].rearrange("(t p) o -> p (t o)", p=128))
                    hidxB_sb = hcon.tile([128, 8], I32)
                    dma(hidxB_sb[:], hidxB[:].rearrange("(t p) o -> p (t o)", p=128))
                    h2T = hact.tile([128, HC * 1024], F32, tag="h2T", name="h2T")

                    def wrh(t, c, pt):
                        nc.scalar.copy(h2T[:, c * 1024 + t * 128: c * 1024 + (t + 1) * 128], pt[:])

                    gather_transpose_T(hsc, wrh, h_ag[1], hidxF_sb, 8, HC,
                                       extra=(h_ag[1], hidxB_sb, HL))
                    dense_b_c = load_cols(hcon, dense_b, ZC, "c_db")
                    normg_c = load_cols(hcon, normg, ZC, "c_ng")
                    normb_c = load_cols(hcon, normb, ZC, "c_nb")
                    pool_b1_c = load_cols(hcon, pool_b1, ZC, "c_pb1")
                    z0 = hact.tile([128, ZC * 1024], F32, tag="z0", name="z0")
                    proj_T(z0, h2T, dense_w, HC, ZC, 1024, bias_col=dense_b_c, func=AF.Relu)
                    zT = hact.tile([128, ZC * 1024], F32, tag="zT", name="zT")
                    ln_T(hsc, zT, z0, ZC, 1024, normg_c, normb_c, HL)
                    clsw_sb = hcon.tile([128, ZC * NUM_LABELS], F32)
                    for k in range(ZC):
                        dma(clsw_sb[:, k * NUM_LABELS:(k + 1) * NUM_LABELS], cls_w[k * 128:(k + 1) * 128, :])
                    clsb_b = hcon.tile([128, NUM_LABELS], F32)
                    dma(clsb_b[:], cls_b[:].to_broadcast([128, NUM_LABELS]))
                    for t in range(8):
                        pe = psp.tile([128, NUM_LABELS], F32, tag="A", bufs=2, name="pe")
                        for k in range(ZC):
                            nc.tensor.matmul(out=pe[:],
                                             lhsT=zT[:, k * 1024 + t * 128: k * 1024 + (t + 1) * 128],
                                             rhs=clsw_sb[:, k * NUM_LABELS:(k + 1) * NUM_LABELS],
                                             start=(k == 0), stop=(k == ZC - 1))
                        esb = hsc.tile([128, NUM_LABELS], F32, tag="esb", name="esb")
                        nc.vector.tensor_add(esb[:], pe[:], clsb_b[:])
                        dma(emis_d[t * 128:(t + 1) * 128, :], esb[:])
                        dma(out_emis[t * 128:(t + 1) * 128, :], esb[:])
                    znat = hact.tile([128, 8 * HL], F32, tag="znat", name="znat")
                    for t in range(8):
                        for c in range(ZC):
                            pt = psp.tile([128, 128], F32, tag="A", bufs=2, name="pt")
                            nc.tensor.transpose(out=pt[:], in_=zT[:, c * 1024 + t * 128: c * 1024 + (t + 1) * 128],
                                                identity=ident[:])
                            nc.scalar.copy(znat[:, t * HL + c * 128: t * HL + (c + 1) * 128], pt[:])
                    sentT = hsc.tile([128, ZC * 4], F32, tag="sentT", name="sentT")
                    for s in range(4):
                        wrow = rows.tile([1, 256], F32, tag="wrow", name="wrow")
                        pw = psp.tile([1, 256], F32, tag="B", bufs=6, name="pw")
                        for qc in range(2):
                            pS = psp.tile([128, 256], F32, tag="B", bufs=6, name="pSz")
                            for k in range(ZC):
                                nc.tensor.matmul(
                                    out=pS[:],
                                    lhsT=zT[:, k * 1024 + s * 256 + qc * 128: k * 1024 + s * 256 + (qc + 1) * 128],
                                    rhs=zT[:, k * 1024 + s * 256: k * 1024 + (s + 1) * 256],
                                    start=(k == 0), stop=(k == ZC - 1))
                            nm = hsc.tile([128, 1], F32, tag="atts", name="nmz")
                            nc.vector.reduce_max(nm[:], pS[:], axis=AX, negate=True)
                            ex = hsc.tile([128, 256], F32, tag="attx", name="exz")
                            sume = hsc.tile([128, 1], F32, tag="atts", name="sumez")
                            nc.scalar.activation(ex[:], pS[:], AF.Exp, bias=nm[:, 0:1],
                                                 scale=1.0, accum_out=sume[:, 0:1])
                            rs = hsc.tile([128, 1], F32, tag="atts", name="rsz")
                            nc.vector.reciprocal(rs[:], sume[:])
                            nc.vector.tensor_scalar(ex[:], ex[:], rs[:, 0:1], None, op0=ALU.mult)
                            nc.tensor.matmul(out=pw[:], lhsT=ones_col[:], rhs=ex[:],
                                             start=(qc == 0), stop=(qc == 1))
                            if qc == 1:
                                nc.scalar.copy(wrow[:], pw[:])
                        wcol = hsc.tile([128, 2], F32, tag="wcol", name="wcol")
                        for kt in range(2):
                            pt = psp.tile([128, 128], F32, tag="A", bufs=2, name="ptw")
                            nc.tensor.transpose(out=pt[:, 0:1], in_=wrow[:, kt * 128:(kt + 1) * 128],
                                                identity=ident[:1, :1])
                            nc.scalar.copy(wcol[:, kt:kt + 1], pt[:, 0:1])
                        for n in range(ZC):
                            psn = psp.tile([128, 1], F32, tag="A", bufs=2, name="psn")
                            for kt in range(2):
                                nc.tensor.matmul(
                                    out=psn[:],
                                    lhsT=znat[:, (2 * s + kt) * HL + n * 128: (2 * s + kt) * HL + (n + 1) * 128],
                                    rhs=wcol[:, kt:kt + 1],
                                    start=(kt == 0), stop=(kt == 1))
                            nc.scalar.mul(sentT[:, n * 4 + s: n * 4 + s + 1], psn[:], 1.0 / 256.0)
                    y1p = hsc.tile([128, ZC * 4], F32, tag="y1p", name="y1p")
                    for n in range(ZC):
                        p = psp.tile([128, 4], F32, tag="A", bufs=2, name="pl1")
                        for k in range(ZC):
                            w = wt.tile([128, 128], F32, tag="w", name="pw1")
                            dma(w[:], pool_w1[k * 128:(k + 1) * 128, n * 128:(n + 1) * 128])
                            nc.tensor.matmul(out=p[:], lhsT=w[:], rhs=sentT[:, k * 4:(k + 1) * 4],
                                             start=(k == 0), stop=(k == ZC - 1))
                        nc.scalar.activation(y1p[:, n * 4:(n + 1) * 4], p[:], AF.Relu,
                                             bias=pool_b1_c[:, n:n + 1], scale=1.0)
                    w2sb = hcon.tile([128, ZC * 2], F32)
                    for k in range(ZC):
                        dma(w2sb[:, k * 2:(k + 1) * 2], pool_w2[k * 128:(k + 1) * 128, :])
                    plg = psp.tile([2, 4], F32, tag="A", bufs=2, name="plg")
                    for k in range(ZC):
                        nc.tensor.matmul(out=plg[:], lhsT=w2sb[:, k * 2:(k + 1) * 2],
                                         rhs=y1p[:, k * 4:(k + 1) * 4],
                                         start=(k == 0), stop=(k == ZC - 1))
                    pb2 = hcon.tile([2, 1], F32)
                    dma(pb2[:], pool_b2[:])
                    lgT = hsc.tile([2, 4], F32, tag="lgT", name="lgT")
                    nc.scalar.activation(lgT[:], plg[:], AF.Identity, bias=pb2[:, 0:1], scale=1.0)
                    dma(slg_d[:].rearrange("s p -> p s"), lgT[:])
                    dma(out_slg[:].rearrange("s p -> p s"), lgT[:])
                    nc.gpsimd.collective_compute("AllGather", ALU.bypass, replica_groups=GRP,
                                                 ins=[slg_d[:]], outs=[slg_ag[:]])
                    nc.gpsimd.collective_compute("AllGather", ALU.bypass, replica_groups=GRP,
                                                 ins=[emis_d[:]], outs=[emis_ag[:]])

                    # ---------------- full-batch CRF ----------------
                    B = BATCH
                    emis = hcon.tile([B, SEQ * 5], F32)
                    dma(emis[:], emis_ag[:].rearrange("(b t) j -> b (t j)", b=B))
                    trans_b = hcon.tile([B, 25], F32)
                    dma(trans_b[:], crf_trans[:].to_broadcast([B, 25]))
                    start_b = hcon.tile([B, 5], F32)
                    dma(start_b[:], crf_start[:].to_broadcast([B, 5]))
                    end_b = hcon.tile([B, 5], F32)
                    dma(end_b[:], crf_end[:].to_broadcast([B, 5]))
                    iota5_b = hcon.tile([B, 5], F32)
                    dma(iota5_b[:], iota5[:].to_broadcast([B, 5]))
                    iota5m8_b = hcon.tile([B, 5], F32)
                    dma(iota5m8_b[:], iota5m8[:].to_broadcast([B, 5]))
                    iota2_b = hcon.tile([B, 2], F32)
                    dma(iota2_b[:], iota2[:].to_broadcast([B, 2]))
                    lab_i = hcon.tile([B, SEQ], I32)
                    dma(lab_i[:], labels_full[:])
                    lab_f = hcon.tile([B, SEQ], F32)
                    nc.vector.tensor_copy(lab_f[:], lab_i[:])
                    mask_i = hcon.tile([B, SEQ], I32)
                    dma(mask_i[:], mask_full[:])
                    mask_f = hcon.tile([B, SEQ], F32)
                    nc.vector.tensor_copy(mask_f[:], mask_i[:])
                    slab_i = hcon.tile([B, 1], I32)
                    dma(slab_i[:], slab_full[:])
                    slab_f = hcon.tile([B, 1], F32)
                    nc.vector.tensor_copy(slab_f[:], slab_i[:])

                    trans_ij = trans_b[:].rearrange("b (i j) -> b i j", j=5)

                    def bcast_ij(ap):
                        return ap.rearrange("b (i o) -> b i o", o=1).to_broadcast([B, 5, 5])

                    # ---- logZ forward scan ----
                    score = hsc.tile([B, 5], F32, tag="lzsc", name="score0")
                    nc.vector.tensor_add(score[:], start_b[:], emis[:, 0:5])
                    for t in range(1, SEQ):
                        cand = hsc.tile([B, 25], F32, tag="cand", name="cand")
                        nc.vector.tensor_tensor(cand[:].rearrange("b (i j) -> b i j", j=5),
                                                bcast_ij(score[:]), trans_ij, op=ALU.add)
                        nm = hsc.tile([B, 1], F32, tag="lzs1", name="nmt")
                        nc.vector.reduce_max(nm[:], cand[:], axis=AX, negate=True)
                        ex = hsc.tile([B, 25], F32, tag="cexp", name="ext")
                        nc.scalar.activation(ex[:], cand[:], AF.Exp, bias=nm[:, 0:1], scale=1.0)
                        sj = hsc.tile([B, 5], F32, tag="sj", name="sj")
                        nc.vector.tensor_add(sj[:], ex[:, 0:5], ex[:, 5:10])
                        nc.vector.tensor_add(sj[:], sj[:], ex[:, 10:15])
                        nc.vector.tensor_add(sj[:], sj[:], ex[:, 15:20])
                        nc.vector.tensor_add(sj[:], sj[:], ex[:, 20:25])
                        lg = hsc.tile([B, 5], F32, tag="lgg", name="lg")
                        nc.scalar.activation(lg[:], sj[:], AF.Ln)
                        score2 = hsc.tile([B, 5], F32, tag="lzsc", name="score")
                        nc.vector.scalar_tensor_tensor(score2[:], lg[:], nm[:, 0:1],
                                                       emis[:, t * 5:(t + 1) * 5],
                                                       op0=ALU.subtract, op1=ALU.add)
                        score = score2
                    fin = hsc.tile([B, 5], F32, tag="fin", name="fin")
                    nc.vector.tensor_add(fin[:], score[:], end_b[:])
                    nmf = hsc.tile([B, 1], F32, tag="lzs1", name="nmf")
                    nc.vector.reduce_max(nmf[:], fin[:], axis=AX, negate=True)
                    exf = hsc.tile([B, 5], F32, tag="fin2", name="exf")
                    sef = hsc.tile([B, 1], F32, tag="lzs2", name="sef")
                    nc.scalar.activation(exf[:], fin[:], AF.Exp, bias=nmf[:, 0:1], scale=1.0,
                                         accum_out=sef[:, 0:1])
                    logz = hsc.tile([B, 1], F32, tag="logz", name="logz")
                    nc.scalar.activation(logz[:], sef[:], AF.Ln)
                    nc.vector.tensor_sub(logz[:], logz[:], nmf[:])

                    # ---- gold path score ----
                    oh = hcon.tile([B, SEQ * 5], F32)
                    oh3 = oh[:].rearrange("b (t j) -> b t j", j=5)
                    nc.vector.tensor_tensor(
                        oh3,
                        lab_f[:].rearrange("b (t o) -> b t o", o=1).to_broadcast([B, SEQ, 5]),
                        iota5_b[:].rearrange("b (o j) -> b o j", o=1).to_broadcast([B, SEQ, 5]),
                        op=ALU.is_equal)
                    esel = hsc.tile([B, SEQ], F32, tag="esel", name="esel")
                    prod = hcon.tile([B, SEQ * 5], F32)
                    nc.vector.tensor_mul(prod[:], oh[:], emis[:])
                    nc.vector.reduce_sum(esel[:].rearrange("b (t o) -> b t o", o=1),
                                         prod[:].rearrange("b (t j) -> b t j", j=5), axis=AX)
                    trtmp = hcon.tile([B, (SEQ - 1) * 5], F32)
                    nc.vector.memset(trtmp[:], 0.0)
                    tmp_i = hsc.tile([B, (SEQ - 1) * 5], F32, tag="tmpi", name="tmpi")
                    for i in range(5):
                        nc.vector.tensor_tensor(
                            tmp_i[:].rearrange("b (t j) -> b t j", j=5),
                            oh3[:, 0:SEQ - 1, i:i + 1].to_broadcast([B, SEQ - 1, 5]),
                            trans_b[:, i * 5:(i + 1) * 5].rearrange("b (o j) -> b o j", o=1).to_broadcast([B, SEQ - 1, 5]),
                            op=ALU.mult)
                        nc.vector.tensor_add(trtmp[:], trtmp[:], tmp_i[:])
                    trsel = hsc.tile([B, SEQ - 1], F32, tag="trsel", name="trsel")
                    nc.vector.tensor_mul(tmp_i[:], trtmp[:], oh[:, 5:])
                    nc.vector.reduce_sum(trsel[:].rearrange("b (t o) -> b t o", o=1),
                                         tmp_i[:].rearrange("b (t j) -> b t j", j=5), axis=AX)
                    st0 = hsc.tile([B, 1], F32, tag="st0", name="st0")
                    t5 = hsc.tile([B, 5], F32, tag="t5", name="t5")
                    nc.vector.tensor_mul(t5[:], oh[:, 0:5], start_b[:])
                    nc.vector.reduce_sum(st0[:], t5[:], axis=AX)
                    t5b = hsc.tile([B, 5], F32, tag="t5", name="t5b")
                    nc.vector.tensor_mul(t5b[:], oh[:, (SEQ - 1) * 5: SEQ * 5], end_b[:])
                    endt = hsc.tile([B, 1], F32, tag="endt", name="endt")
                    nc.vector.reduce_sum(endt[:], t5b[:], axis=AX)
                    tre = hsc.tile([B, SEQ - 1], F32, tag="tre", name="tre")
                    nc.vector.tensor_add(tre[:], trsel[:], esel[:, 1:])
                    nc.vector.tensor_mul(tre[:], tre[:], mask_f[:, 1:])
                    smid = hsc.tile([B, 1], F32, tag="smid", name="smid")
                    nc.vector.reduce_sum(smid[:], tre[:], axis=AX)
                    gold = hsc.tile([B, 1], F32, tag="gold", name="gold")
                    nc.vector.tensor_add(gold[:], st0[:], smid[:])
                    nc.vector.tensor_add(gold[:], gold[:], endt[:])
                    nc.vector.tensor_add(gold[:], gold[:], esel[:, 0:1])
                    llh = hsc.tile([B, 1], F32, tag="llh", name="llh")
                    nc.vector.tensor_sub(llh[:], gold[:], logz[:])

                    # ---- viterbi ----
                    vsc = hsc.tile([B, 5], F32, tag="vsc", name="vsc0")
                    nc.vector.tensor_add(vsc[:], start_b[:], emis[:, 0:5])
                    hist = hcon.tile([B, (SEQ - 1) * 5], F32)
                    for t in range(1, SEQ):
                        cand = hsc.tile([B, 25], F32, tag="cand", name="vcand")
                        nc.vector.tensor_tensor(cand[:].rearrange("b (i j) -> b i j", j=5),
                                                bcast_ij(vsc[:]), trans_ij, op=ALU.add)
                        m5 = hsc.tile([B, 5], F32, tag="m5", name="m5")
                        nc.vector.tensor_max(m5[:], cand[:, 0:5], cand[:, 5:10])
                        nc.vector.tensor_max(m5[:], m5[:], cand[:, 10:15])
                        nc.vector.tensor_max(m5[:], m5[:], cand[:, 15:20])
                        nc.vector.tensor_max(m5[:], m5[:], cand[:, 20:25])
                        idxe = hsc.tile([B, 5], F32, tag="idxe", name="idxe")
                        eq = hsc.tile([B, 5], F32, tag="eq", name="eq0")
                        nc.vector.tensor_tensor(eq[:], cand[:, 0:5], m5[:], op=ALU.is_equal)
                        nc.vector.tensor_scalar(idxe[:], eq[:], -8.0, None, op0=ALU.mult)
                        for i in range(1, 5):
                            eqi = hsc.tile([B, 5], F32, tag="eq", name="eqi")
                            nc.vector.tensor_tensor(eqi[:], cand[:, i * 5:(i + 1) * 5], m5[:], op=ALU.is_equal)
                            nc.vector.scalar_tensor_tensor(idxe[:], eqi[:], float(i - 8), idxe[:],
                                                           op0=ALU.mult, op1=ALU.min)
                        nc.vector.tensor_scalar(hist[:, (t - 1) * 5: t * 5], idxe[:], 8.0, None, op0=ALU.add)
                        vsc2 = hsc.tile([B, 5], F32, tag="vsc", name="vsc")
                        nc.vector.tensor_add(vsc2[:], m5[:], emis[:, t * 5:(t + 1) * 5])
                        vsc = vsc2
                    fine = hsc.tile([B, 5], F32, tag="fine", name="fine")
                    nc.vector.tensor_add(fine[:], vsc[:], end_b[:])
                    mfin = hsc.tile([B, 1], F32, tag="mfin", name="mfin")
                    nc.vector.reduce_max(mfin[:], fine[:], axis=AX)
                    eqf = hsc.tile([B, 5], F32, tag="eqf", name="eqf")
                    nc.vector.tensor_scalar(eqf[:], fine[:], mfin[:, 0:1], None, op0=ALU.is_equal)
                    ence = hsc.tile([B, 5], F32, tag="ence", name="ence")
                    nc.vector.tensor_mul(ence[:], eqf[:], iota5m8_b[:])
                    cur = hsc.tile([B, 1], F32, tag="cur", name="cur0")
                    nc.vector.tensor_reduce(cur[:], ence[:], axis=AX, op=ALU.min)
                    nc.vector.tensor_scalar(cur[:], cur[:], 8.0, None, op0=ALU.add)
                    preds = hcon.tile([B, SEQ], F32)
                    nc.vector.tensor_copy(preds[:, SEQ - 1: SEQ], cur[:])
                    for t in range(SEQ - 2, -1, -1):
                        ohc = hsc.tile([B, 5], F32, tag="ohc", name="ohc")
                        nc.vector.tensor_scalar(ohc[:], iota5_b[:], cur[:, 0:1], None, op0=ALU.is_equal)
                        nc.vector.tensor_mul(ohc[:], ohc[:], hist[:, t * 5:(t + 1) * 5])
                        cur = hsc.tile([B, 1], F32, tag="cur", name="cur")
                        nc.vector.reduce_sum(cur[:], ohc[:], axis=AX)
                        nc.vector.tensor_copy(preds[:, t:t + 1], cur[:])
                    dma(out_preds[:], preds[:])

                    # ---- sentence CE + loss ----
                    slg_sb = hcon.tile([B, 2], F32)
                    dma(slg_sb[:], slg_ag[:])
                    nm2 = hsc.tile([B, 1], F32, tag="nm2", name="nm2")
                    nc.vector.reduce_max(nm2[:], slg_sb[:], axis=AX, negate=True)
                    ex2 = hsc.tile([B, 2], F32, tag="ex2", name="ex2")
                    se2 = hsc.tile([B, 1], F32, tag="se2", name="se2")
                    nc.scalar.activation(ex2[:], slg_sb[:], AF.Exp, bias=nm2[:, 0:1], scale=1.0,
                                         accum_out=se2[:, 0:1])
                    lse2 = hsc.tile([B, 1], F32, tag="lse2", name="lse2")
                    nc.scalar.activation(lse2[:], se2[:], AF.Ln)
                    nc.vector.tensor_sub(lse2[:], lse2[:], nm2[:])
                    sel2 = hsc.tile([B, 2], F32, tag="sel2", name="sel2")
                    nc.vector.tensor_scalar(sel2[:], iota2_b[:], slab_f[:, 0:1], None, op0=ALU.is_equal)
                    nc.vector.tensor_mul(sel2[:], sel2[:], slg_sb[:])
                    pick = hsc.tile([B, 1], F32, tag="pick", name="pick")
                    nc.vector.reduce_sum(pick[:], sel2[:], axis=AX)
                    ce = hsc.tile([B, 1], F32, tag="ce", name="ce")
                    nc.vector.tensor_sub(ce[:], lse2[:], pick[:])
                    psl = psp.tile([1, 1], F32, tag="A", bufs=2, name="psl")
                    nc.tensor.matmul(out=psl[:], lhsT=ones_col[0:B, :], rhs=llh[:], start=True, stop=True)
                    sllh = hsc.tile([1, 1], F32, tag="sllh", name="sllh")
                    nc.scalar.copy(sllh[:], psl[:])
                    psc_ = psp.tile([1, 1], F32, tag="A", bufs=2, name="psc_")
                    nc.tensor.matmul(out=psc_[:], lhsT=ones_col[0:B, :], rhs=ce[:], start=True, stop=True)
                    sce = hsc.tile([1, 1], F32, tag="sce", name="sce")
                    nc.scalar.copy(sce[:], psc_[:])
                    lossa = hsc.tile([1, 1], F32, tag="lossa", name="lossa")
                    nc.scalar.mul(lossa[:], sllh[:], -0.7 / BATCH)
                    lossb = hsc.tile([1, 1], F32, tag="lossb", name="lossb")
                    nc.scalar.mul(lossb[:], sce[:], 0.3 / BATCH)
                    loss = hsc.tile([1, 1], F32, tag="loss", name="loss")
                    nc.vector.tensor_add(loss[:], lossa[:], lossb[:])
                    dma(out_loss[:], loss[:])

    nc.compile()
    return nc


# =====================================================================
# Host side
# =====================================================================
def _col(v, nch):
    return np.ascontiguousarray(np.asarray(v, np.float32).reshape(nch, 128).T)


def _prep_inmaps(input_ids, attention_mask, labels, sentence_labels, params, nl,
                 run_lstm, run_head):
    p = params
    f32 = lambda a: np.ascontiguousarray(np.asarray(a, np.float32))
    i32 = lambda a: np.ascontiguousarray(np.asarray(a, np.int32))

    common = {}
    common["wemb"] = f32(p["word_emb"])
    common["pos_type"] = f32(np.asarray(p["pos_emb"])[:SEQ] + np.asarray(p["type_emb"])[None, :])
    common["embg"] = f32(p["emb_ln_g"]).reshape(1, D)
    common["embb"] = f32(p["emb_ln_b"]).reshape(1, D)
    for l in range(nl):
        lp = p["layers"][l]
        common[f"L{l}_wq"] = f32(lp["wq"]); common[f"L{l}_wk"] = f32(lp["wk"])
        common[f"L{l}_wv"] = f32(lp["wv"]); common[f"L{l}_wo"] = f32(lp["wo"])
        common[f"L{l}_bqkv"] = np.concatenate(
            [_col(lp["bq"], DC), _col(lp["bk"], DC), _col(lp["bv"], DC)], axis=1)
        common[f"L{l}_bo"] = _col(lp["bo"], DC)
        common[f"L{l}_ln1g"] = _col(lp["ln1_g"], DC); common[f"L{l}_ln1b"] = _col(lp["ln1_b"], DC)
        common[f"L{l}_w1"] = f32(lp["w1"]); common[f"L{l}_b1"] = _col(lp["b1"], FC)
        common[f"L{l}_w2"] = f32(lp["w2"]); common[f"L{l}_b2"] = _col(lp["b2"], DC)
        common[f"L{l}_ln2g"] = _col(lp["ln2_g"], DC); common[f"L{l}_ln2b"] = _col(lp["ln2_b"], DC)
    if run_head:
        common["dense_w"] = f32(p["dense_w"])
        common["dense_b"] = _col(p["dense_b"], ZC)
        common["normg"] = _col(p["norm_g"], ZC); common["normb"] = _col(p["norm_b"], ZC)
        common["cls_w"] = f32(p["cls_w"]); common["cls_b"] = f32(p["cls_b"]).reshape(1, NUM_LABELS)
        common["pool_w1"] = f32(p["pool_w1"]); common["pool_b1"] = _col(p["pool_b1"], ZC)
        common["pool_w2"] = f32(p["pool_w2"])
        common["pool_b2"] = f32(p["pool_b2"]).reshape(2, 1)
        common["crf_start"] = f32(p["crf_start"]).reshape(1, 5)
        common["crf_end"] = f32(p["crf_end"]).reshape(1, 5)
        common["crf_trans"] = f32(p["crf_trans"]).reshape(1, 25)
        common["iota5"] = np.arange(5, dtype=np.float32).reshape(1, 5)
        common["iota5m8"] = (np.arange(5, dtype=np.float32) - 8.0).reshape(1, 5)
        common["iota2"] = np.arange(2, dtype=np.float32).reshape(1, 2)
        common["labels_full"] = i32(labels)
        common["mask_full"] = i32(attention_mask)
        common["slab_full"] = i32(sentence_labels).reshape(BATCH, 1)

    in_maps = []
    ids_np = np.asarray(input_ids, np.int32)
    for c in range(NCORES):
        m = dict(common)
        m["ids"] = ids_np[4 * c:4 * c + 4].reshape(TOK, 1).copy()
        if run_lstm:
            d, g = c // 4, c % 4
            for ll in range(2):
                lp = p["lstm"][ll]["fwd" if d == 0 else "bwd"]
                m[f"wih{ll}"] = np.ascontiguousarray(np.asarray(lp["wih"], np.float32).T)
                whhT = np.asarray(lp["whh"], np.float32).T
                m[f"whh{ll}"] = np.ascontiguousarray(
                    whhT.reshape(3, 128, GC, 128).transpose(1, 0, 2, 3).reshape(128, 3 * G4)
                ).astype(np.float16)
                m[f"lbc{ll}"] = _col(lp["b"], GC)
            tau = np.arange(SEQ)
            tnat = tau if d == 0 else (SEQ - 1 - tau)
            qv = 8 * g + np.arange(LB)
            l1 = (qv[None, :] // 4) * TOK + (qv[None, :] % 4) * SEQ + tnat[:, None]
            m["lidx1"] = i32(l1.reshape(LTOK, 1))
            l2F = (qv[None, :] // 8) * (LB * SEQ) + (qv[None, :] % 8) * SEQ + tnat[:, None]
            l2B = (4 + qv[None, :] // 8) * (LB * SEQ) + (qv[None, :] % 8) * SEQ + (SEQ - 1 - tnat)[:, None]
            m["lidx2F"] = i32(l2F.reshape(LTOK, 1))
            m["lidx2B"] = i32(l2B.reshape(LTOK, 1))
        if run_head:
            t = np.arange(SEQ)
            qv = 4 * c + np.arange(4)
            hF = (qv[:, None] // 8) * (LB * SEQ) + (qv[:, None] % 8) * SEQ + t[None, :]
            hB = (4 + qv[:, None] // 8) * (LB * SEQ) + (qv[:, None] % 8) * SEQ + (SEQ - 1 - t)[None, :]
            m["hidxF"] = i32(hF.reshape(TOK, 1))
            m["hidxB"] = i32(hB.reshape(TOK, 1))
        in_maps.append(m)
    return in_maps


def run_cores(input_ids, attention_mask, labels, sentence_labels, params,
              nl=NL, run_lstm=True, run_head=True, trace=False):
    from concourse.bass_utils import run_bass_kernel_spmd
    key = (nl, run_lstm, run_head)
    if key not in _CACHE:
        _CACHE[key] = _build(nl, run_lstm, run_head)
    nc = _CACHE[key]
    in_maps = _prep_inmaps(input_ids, attention_mask, labels, sentence_labels,
                           params, nl, run_lstm, run_head)
    return run_bass_kernel_spmd(nc, in_maps, list(range(NCORES)), trace=trace)


def kernel(input_ids, attention_mask, labels, sentence_labels, params):
    res = run_cores(input_ids, attention_mask, labels, sentence_labels, params)
    r = res.results
    loss = np.asarray(np.float32(r[0]["out_loss"][0, 0]))
    emissions = np.stack([r[c]["out_emis"].reshape(4, SEQ, NUM_LABELS) for c in range(NCORES)])
    emissions = np.ascontiguousarray(emissions.reshape(BATCH, SEQ, NUM_LABELS))
    preds = np.rint(r[0]["out_preds"]).astype(np.int32)
    sent_logits = np.concatenate([r[c]["out_slg"] for c in range(NCORES)], axis=0)
    return loss, emissions, preds, sent_logits
